# revision 12
# baseline (speedup 1.0000x reference)
"""MemEffEquivariantAttention TRN2 Bass kernel (transposed-scores flow, v3).

Sharding: 8 cores = 4 batches x 2 query-token halves (fully data-parallel,
no collectives).

Scores are computed TRANSPOSED (wT[s, t] = kT_chunk.T @ qT) so the
attention probabilities already have s on partitions and feed the attn
matmul directly -- no transpose of probabilities is ever materialized.
Z (softmax denominator, per (head, t)) is recovered with ones-vector
matmuls over the s-partitions.

v3 changes driven by the previous trace (137us: PE busy 86us with
439 small matmuls; DVE 81us of which 28us was `reciprocal` on
single-partition [1,256] tiles; 57us of HWDGE dma_start issue cost
spread over SP/ACT/Pool):
  - all heads use host-side exp(bias) (eb): the bias identity matmuls
    are gone (-32 matmuls); bias add is a bf16 DVE multiply.
  - 1/Z is computed as exp(-ln(Z)) on the ACT engine (ln and exp live in
    the same activation table set -> no table reloads), broadcast across
    partitions on the otherwise-idle gpsimd engine.  No DVE reciprocal.
  - q/k/bias loads are batched per group of 4 heads (3 dma_starts per
    group instead of 12), v/law/WT/out merged; X-stash DMAs split
    across the sync and gpsimd rings.
  - exp uses a constant -40 bias (softmax shift, folded out exactly by
    Z) to keep e/m0 in comfortable bf16/f32 range.
  - q/k fp16, output bf16 (upcast on host).
"""
import sys
sys.path.insert(0, "/opt/trn_rl_repo")

import numpy as np
import ml_dtypes

import concourse.bacc as bacc
import concourse.tile as tile
from concourse import mybir
from concourse.bass_utils import run_bass_kernel_spmd

F32 = mybir.dt.float32
F16 = mybir.dt.float16
BF16 = mybir.dt.bfloat16
AF = mybir.ActivationFunctionType

B, T, P, HID = 4, 512, 3, 512
HD, H = 32, 16
EXP, S = 512, 1024
TQ = 256            # query tokens per core
EPS = 1e-3
CUTOFF = 1e-5
NEG = -1e30
D = P * HD          # 96, per-head feature dim
SHIFT = -40.0       # constant softmax shift, cancels exactly via Z

_prog_cache = {}


def _build_program():
    nc = bacc.Bacc("TRN2", target_bir_lowering=False, debug=False)

    qT_d = nc.dram_tensor("qT", [H, D, TQ], F16, kind="ExternalInput").ap()
    kT_d = nc.dram_tensor("kT", [H, D, S], F16, kind="ExternalInput").ap()
    vS_d = nc.dram_tensor("vS", [128, 8, H, D], BF16, kind="ExternalInput").ap()
    # eb = exp(masked bias)^T, [H, s(part,chunk), t]
    eb_d = nc.dram_tensor("eb", [H, 128, 2, 4 * TQ], BF16, kind="ExternalInput").ap()
    lawT_d = nc.dram_tensor("lawT", [128, 2, 4 * TQ], BF16, kind="ExternalInput").ap()
    WT_d = nc.dram_tensor("WT", [128, 4, HID], BF16, kind="ExternalInput").ap()
    ones128_d = nc.dram_tensor("ones128", [128, 1], BF16, kind="ExternalInput").ap()
    ones96_d = nc.dram_tensor("ones96", [D, 1], F32, kind="ExternalInput").ap()
    out_d = nc.dram_tensor("out", [TQ, P, HID], BF16, kind="ExternalOutput").ap()

    with tile.TileContext(nc) as tc:
        with tc.tile_pool(name="const", bufs=1) as cp, \
             tc.tile_pool(name="kq", bufs=2) as kq, \
             tc.tile_pool(name="ebp", bufs=2) as ebp, \
             tc.tile_pool(name="eu", bufs=2) as eu, \
             tc.tile_pool(name="work", bufs=3) as wp, \
             tc.tile_pool(name="psw", bufs=2, space="PSUM") as psw, \
             tc.tile_pool(name="psa", bufs=2, space="PSUM") as psa, \
             tc.tile_pool(name="psz", bufs=2, space="PSUM") as psz:

            # ---- constants ----
            vS_t = cp.tile([128, 8, H, D], BF16, tag="vS")
            lawT_t = cp.tile([128, 2, 4 * TQ], BF16, tag="lawT")
            WT_t = cp.tile([128, 4, HID], BF16, tag="WT")
            ones128_t = cp.tile([128, 1], BF16, tag="o128")
            ones96_t = cp.tile([D, 1], F32, tag="o96")
            eps_t = cp.tile([128, 1], F32, tag="eps")
            shift_t = cp.tile([128, 1], F32, tag="shift")
            zero_t = cp.tile([128, 1], F32, tag="zero")
            X_t = cp.tile([128, P, 4, TQ], BF16, tag="X")
            sqacc_t = cp.tile([D, TQ], F32, tag="sqacc")
            nc.vector.memset(eps_t[:], EPS)
            nc.vector.memset(shift_t[:], SHIFT)
            nc.vector.memset(zero_t[:], 0.0)

            kT_tiles, qT_tiles, eb_tiles = {}, {}, {}
            m0_tiles, u_tiles, z_tiles, at_tiles = {}, {}, {}, {}

            def emit_loads(g):
                """Load kT/qT/eb for the 4 heads of group g (3 DMAs)."""
                kT_t = kq.tile([D, 4, S], F16, tag="kT", name=f"kT_{g}")
                qT_t = kq.tile([D, 4, TQ], F16, tag="qT", name=f"qT_{g}")
                eb_t = ebp.tile([128, 4, 2, 4 * TQ], BF16, tag="eb",
                                name=f"eb_{g}")
                hs = slice(4 * g, 4 * g + 4)
                nc.sync.dma_start(out=kT_t[:],
                                  in_=kT_d[hs].rearrange("h d s -> d h s"))
                nc.sync.dma_start(out=qT_t[:],
                                  in_=qT_d[hs].rearrange("h d t -> d h t"))
                nc.sync.dma_start(out=eb_t[:],
                                  in_=eb_d[hs].rearrange("h p f x -> p h f x"))
                kT_tiles[g], qT_tiles[g], eb_tiles[g] = kT_t, qT_t, eb_t

            def emit_scores(h):
                g, h4 = h // 4, h % 4
                kT_t, qT_t, eb_t = kT_tiles[g], qT_tiles[g], eb_tiles[g]
                e_t = eu.tile([128, 2, 4 * TQ], BF16, tag="e", name=f"e_{h}")
                m0_t = eu.tile([128, 2, 4 * TQ], BF16, tag="m0",
                               name=f"m0_{h}")
                u_t = eu.tile([128, 2, 4 * TQ], BF16, tag="u", name=f"u_{h}")
                for hf in range(2):
                    w_ps = psw.tile([128, 4 * TQ], F32, tag="w",
                                    name=f"w_{h}_{hf}")
                    for sc4 in range(4):
                        sc = 4 * hf + sc4
                        nc.tensor.matmul(w_ps[:, sc4 * TQ:(sc4 + 1) * TQ],
                                         kT_t[:, h4, sc * 128:(sc + 1) * 128],
                                         qT_t[:, h4, :],
                                         start=True, stop=True,
                                         skip_group_check=True)
                    nc.scalar.activation(e_t[:, hf, :], w_ps[:], AF.Exp,
                                         bias=shift_t[:])
                    nc.vector.tensor_mul(m0_t[:, hf, :], e_t[:, hf, :],
                                         eb_t[:, h4, hf, :])
                    nc.vector.tensor_mul(u_t[:, hf, :], m0_t[:, hf, :],
                                         lawT_t[:, hf, :])
                m0_tiles[h], u_tiles[h] = m0_t, u_t

            def emit_post_pe(h):
                m0_t, u_t = m0_tiles[h], u_tiles[h]
                z_ps = psz.tile([1, TQ], F32, tag="z", name=f"z_{h}")
                for sc in range(8):
                    nc.tensor.matmul(z_ps[:], ones128_t[:],
                                     m0_t[:, sc // 4,
                                          (sc % 4) * TQ:(sc % 4 + 1) * TQ],
                                     start=(sc == 0), stop=(sc == 7))
                at_ps = psa.tile([D, TQ], F32, tag="at", name=f"at_{h}")
                for sc in range(8):
                    nc.tensor.matmul(at_ps[:], vS_t[:, sc, h, :],
                                     u_t[:, sc // 4,
                                         (sc % 4) * TQ:(sc % 4 + 1) * TQ],
                                     start=(sc == 0), stop=(sc == 7))
                z_tiles[h], at_tiles[h] = z_ps, at_ps

            def emit_post_rest(h):
                del m0_tiles[h], u_tiles[h]
                at_ps = at_tiles.pop(h)
                # 1/Z = exp(-ln Z): ACT-only (ln+exp share one table set)
                lnz_t = wp.tile([1, TQ], F32, tag="lnz", name=f"lnz_{h}")
                nc.scalar.activation(lnz_t[:], z_tiles.pop(h)[:], AF.Ln,
                                     bias=zero_t[0:1, :])
                lnzb_t = wp.tile([D, TQ], F32, tag="lnzb", name=f"lnzb_{h}")
                nc.gpsimd.partition_broadcast(lnzb_t[:], lnz_t[:])
                rzb_t = wp.tile([D, TQ], F32, tag="rzb", name=f"rzb_{h}")
                nc.scalar.activation(rzb_t[:], lnzb_t[:], AF.Exp,
                                     bias=zero_t[0:D, :], scale=-1.0)
                at_sb = wp.tile([D, TQ], BF16, tag="atsb", name=f"atsb_{h}")
                nc.vector.tensor_mul(at_sb[:], at_ps[:], rzb_t[:])
                # stash into X[(h%4)*32+j, p, h//4, t] for out_proj lhsT
                for p, eng in ((0, nc.sync), (1, nc.sync), (2, nc.gpsimd)):
                    eng.dma_start(
                        out=X_t[(h % 4) * 32:(h % 4 + 1) * 32, p, h // 4, :],
                        in_=at_sb[p * 32:(p + 1) * 32, :])
                # sumsq accumulate on DVE (f32 accumulator)
                if h == 0:
                    nc.vector.tensor_mul(sqacc_t[:], at_sb[:], at_sb[:])
                else:
                    sq_t = wp.tile([D, TQ], BF16, tag="sq")
                    nc.vector.tensor_mul(sq_t[:], at_sb[:], at_sb[:])
                    nc.vector.tensor_add(sqacc_t[:], sqacc_t[:], sq_t[:])

            # ---- main loop, software-pipelined by one head ----
            emit_loads(0)
            nc.scalar.dma_start(out=lawT_t[:], in_=lawT_d)
            nc.scalar.dma_start(out=ones128_t[:], in_=ones128_d)
            nc.scalar.dma_start(out=ones96_t[:], in_=ones96_d)
            emit_loads(1)
            for half in range(2):
                nc.scalar.dma_start(out=vS_t[:, 4 * half:4 * half + 4, :, :],
                                    in_=vS_d[:, 4 * half:4 * half + 4, :, :])
            nc.scalar.dma_start(out=WT_t[:], in_=WT_d)

            for h in range(H):
                if h % 4 == 0 and h // 4 + 2 < 4:
                    emit_loads(h // 4 + 2)
                if h >= 1:
                    emit_post_pe(h - 1)
                    emit_post_rest(h - 1)
                emit_scores(h)
            emit_post_pe(H - 1)
            emit_post_rest(H - 1)

            # ---- inv = 1/sqrt(mean+eps), out_proj, scale, store ----
            ss_ps = [psz.tile([128, 1], F32, tag="z", name=f"ss{tb}")
                     for tb in range(2)]
            for tb in range(2):
                nc.tensor.matmul(ss_ps[tb][:],
                                 sqacc_t[:, tb * 128:(tb + 1) * 128],
                                 ones96_t[:], start=True, stop=True)
            inv_t = []
            for tb in range(2):
                tmp_t = wp.tile([128, 1], F32, tag=f"tmp{tb}")
                nc.scalar.activation(tmp_t[:], ss_ps[tb][:], AF.Sqrt,
                                     scale=1.0 / HID, bias=eps_t[:])
                iv = wp.tile([128, 1], F32, tag=f"inv{tb}")
                nc.vector.reciprocal(iv[:], tmp_t[:])
                inv_t.append(iv)

            for tb in range(2):
                o_sb = wp.tile([128, P, HID], BF16, tag="osb",
                               name=f"osb_{tb}")
                for p in range(P):
                    o_ps = psw.tile([128, HID], F32, tag="w",
                                    name=f"o_{p}_{tb}")
                    for ci in range(4):
                        nc.tensor.matmul(o_ps[:],
                                         X_t[:, p, ci, tb * 128:(tb + 1) * 128],
                                         WT_t[:, ci, :],
                                         start=(ci == 0), stop=(ci == 3))
                    nc.vector.tensor_scalar_mul(o_sb[:, p, :], o_ps[:],
                                                inv_t[tb][:])
                nc.sync.dma_start(
                    out=out_d[tb * 128:(tb + 1) * 128, :, :], in_=o_sb[:])

    nc.compile()
    return nc


def _get_program():
    if "nc" not in _prog_cache:
        _prog_cache["nc"] = _build_program()
    return _prog_cache["nc"]


def _prepare_in_maps(q, k, v, attn_bias, key_padding_mask, outcell_index,
                     local_attention_weight, expand_mask, out_proj_weight,
                     attn_ln_weight):
    q = np.asarray(q, dtype=np.float32)
    k = np.asarray(k, dtype=np.float32)
    v = np.asarray(v, dtype=np.float32)
    attn_bias = np.asarray(attn_bias, dtype=np.float32)
    kpm = np.asarray(key_padding_mask)
    idx = np.asarray(outcell_index).astype(np.int64)
    law = np.asarray(local_attention_weight, dtype=np.float32)
    emask = np.asarray(expand_mask)
    W = np.asarray(out_proj_weight, dtype=np.float32)
    lnw = np.asarray(attn_ln_weight, dtype=np.float32)

    WT = np.ascontiguousarray((W * lnw[None, :]).T)  # [hid, o], ln folded
    ones128_np = np.ones((128, 1), dtype=ml_dtypes.bfloat16)
    ones96_np = np.ones((D, 1), dtype=np.float32)

    in_maps = []
    for c in range(8):
        b, th = c // 2, c % 2
        tsl = slice(th * TQ, (th + 1) * TQ)

        # kT [H, 96, S]: kf[s, p, h*32+hd] with s-expansion host-gathered
        kf = np.concatenate([k[b], k[b][idx[b]]], axis=0)  # [S, P, HID]
        kT = kf.reshape(S, P, H, HD).transpose(2, 1, 3, 0).reshape(H, D, S)
        qT = q[b, tsl].reshape(TQ, P, H, HD).transpose(2, 1, 3, 0) \
            .reshape(H, D, TQ)

        # vS [128, 8, H, 96]: vS[part, sc, h, (p,hd)] = vf[sc*128+part, ...]
        vf = np.concatenate([v[b], v[b][idx[b]]], axis=0)  # [S, P, HID]
        vS = vf.reshape(8, 128, P, H, HD).transpose(1, 0, 3, 2, 4) \
            .reshape(128, 8, H, D)

        # masked bias [H, 256, S]
        bias_c = np.ascontiguousarray(attn_bias[b, :, tsl, :])
        kpmS = np.concatenate([kpm[b], emask[b]])           # [S]
        if kpmS.any():
            bias_c[:, :, kpmS] = NEG
        cut = law[b, tsl] <= CUTOFF                         # [256, S]
        if cut.any():
            bias_c[:, cut] = NEG
        # exp, transpose to [H, S, 256] -> [H, 128, 2, 1024]
        ebT = np.exp(bias_c.transpose(0, 2, 1)).reshape(H, 8, 128, TQ) \
            .transpose(0, 2, 1, 3).reshape(H, 128, 2, 4 * TQ)

        lawT = law[b, tsl].T.reshape(8, 128, TQ).transpose(1, 0, 2) \
            .reshape(128, 2, 4 * TQ)

        in_maps.append(dict(
            qT=qT.astype(np.float16),
            kT=np.ascontiguousarray(kT).astype(np.float16),
            vS=np.ascontiguousarray(vS).astype(ml_dtypes.bfloat16),
            eb=np.ascontiguousarray(ebT).astype(ml_dtypes.bfloat16),
            lawT=np.ascontiguousarray(lawT).astype(ml_dtypes.bfloat16),
            WT=WT.reshape(4, 128, HID).transpose(1, 0, 2).astype(
                ml_dtypes.bfloat16).copy(),
            ones128=ones128_np,
            ones96=ones96_np,
        ))
    return in_maps


def kernel(**inputs):
    in_maps = _prepare_in_maps(**inputs)
    nc = _get_program()
    res = run_bass_kernel_spmd(nc, in_maps, list(range(8)))

    out = np.empty((B, T, P, HID), dtype=np.float32)
    for c in range(8):
        b, th = c // 2, c % 2
        out[b, th * TQ:(th + 1) * TQ] = res.results[c]["out"].astype(np.float32)
    return out


# revision 17
# speedup vs baseline: 1.0988x; 1.0988x over previous
"""MemEffEquivariantAttention TRN2 Bass kernel (transposed-scores flow, v3).

Sharding: 8 cores = 4 batches x 2 query-token halves (fully data-parallel,
no collectives).

Scores are computed TRANSPOSED (wT[s, t] = kT_chunk.T @ qT) so the
attention probabilities already have s on partitions and feed the attn
matmul directly -- no transpose of probabilities is ever materialized.
Z (softmax denominator, per (head, t)) is recovered with ones-vector
matmuls over the s-partitions.

v4: Z-normalization is applied LATE: the main loop stashes UNNORMALIZED
attn rows into X (plain ACT copy), Z rows are collected per group of 4
heads into an SBUF tile, one DVE reciprocal per group computes rz, gpsimd
broadcasts it into an rzX tile, and 3 DVE multiplies per group normalize
X in place (sumsq follows on the normalized X).  This removes the
per-head reciprocal (1.7us each) / ACT ln-exp chain (which thrashed
activation tables at 1.3us per reload) from the critical path entirely.

v3 changes driven by the previous trace (137us: PE busy 86us with
439 small matmuls; DVE 81us of which 28us was `reciprocal` on
single-partition [1,256] tiles; 57us of HWDGE dma_start issue cost
spread over SP/ACT/Pool):
  - all heads use host-side exp(bias) (eb): the bias identity matmuls
    are gone (-32 matmuls); bias add is a bf16 DVE multiply.
  - 1/Z is computed as exp(-ln(Z)) on the ACT engine (ln and exp live in
    the same activation table set -> no table reloads), broadcast across
    partitions on the otherwise-idle gpsimd engine.  No DVE reciprocal.
  - q/k/bias loads are batched per group of 4 heads (3 dma_starts per
    group instead of 12), v/law/WT/out merged; X-stash DMAs split
    across the sync and gpsimd rings.
  - exp uses a constant -40 bias (softmax shift, folded out exactly by
    Z) to keep e/m0 in comfortable bf16/f32 range.
  - q/k fp16, output bf16 (upcast on host).
"""
import sys
sys.path.insert(0, "/opt/trn_rl_repo")

import numpy as np
import ml_dtypes

import concourse.bacc as bacc
import concourse.tile as tile
from concourse import mybir
from concourse.bass_utils import run_bass_kernel_spmd

F32 = mybir.dt.float32
F16 = mybir.dt.float16
BF16 = mybir.dt.bfloat16
AF = mybir.ActivationFunctionType

B, T, P, HID = 4, 512, 3, 512
HD, H = 32, 16
EXP, S = 512, 1024
TQ = 256            # query tokens per core
EPS = 1e-3
CUTOFF = 1e-5
NEG = -1e30
D = P * HD          # 96, per-head feature dim
SHIFT = -40.0       # constant softmax shift, cancels exactly via Z

_prog_cache = {}


def _build_program():
    nc = bacc.Bacc("TRN2", target_bir_lowering=False, debug=False)

    qT_d = nc.dram_tensor("qT", [H, D, TQ], F16, kind="ExternalInput").ap()
    kT_d = nc.dram_tensor("kT", [H, D, S], F16, kind="ExternalInput").ap()
    vS_d = nc.dram_tensor("vS", [128, 8, H, D], BF16, kind="ExternalInput").ap()
    # eb = exp(masked bias)^T, [H, s(part,chunk), t]
    eb_d = nc.dram_tensor("eb", [H, 128, 2, 4 * TQ], BF16, kind="ExternalInput").ap()
    lawT_d = nc.dram_tensor("lawT", [128, 2, 4 * TQ], BF16, kind="ExternalInput").ap()
    WT_d = nc.dram_tensor("WT", [128, 4, HID], BF16, kind="ExternalInput").ap()
    ones128_d = nc.dram_tensor("ones128", [128, 1], BF16, kind="ExternalInput").ap()
    E4_d = nc.dram_tensor("E4", [128, 4, 4], BF16, kind="ExternalInput").ap()
    ones128f_d = nc.dram_tensor("ones128f", [128, 1], F32, kind="ExternalInput").ap()
    out_d = nc.dram_tensor("out", [TQ, P, HID], BF16, kind="ExternalOutput").ap()

    with tile.TileContext(nc) as tc:
        with tc.tile_pool(name="const", bufs=1) as cp, \
             tc.tile_pool(name="kq", bufs=2) as kq, \
             tc.tile_pool(name="ebp", bufs=2) as ebp, \
             tc.tile_pool(name="eu", bufs=2) as eu, \
             tc.tile_pool(name="work", bufs=3) as wp, \
             tc.tile_pool(name="psw", bufs=2, space="PSUM") as psw, \
             tc.tile_pool(name="psa", bufs=2, space="PSUM") as psa, \
             tc.tile_pool(name="psz", bufs=2, space="PSUM") as psz:

            # ---- constants ----
            vS_t = cp.tile([128, 8, H, D], BF16, tag="vS")
            lawT_t = cp.tile([128, 2, 4 * TQ], BF16, tag="lawT")
            WT_t = cp.tile([128, 4, HID], BF16, tag="WT")
            ones128_t = cp.tile([128, 1], BF16, tag="o128")
            E4_t = cp.tile([128, 4, 4], BF16, tag="E4")
            ones128f_t = cp.tile([128, 1], F32, tag="o128f")
            eps_t = cp.tile([128, 1], F32, tag="eps")
            shift_t = cp.tile([128, 1], F32, tag="shift")
            X_t = cp.tile([128, P, 4, TQ], BF16, tag="X")
            rzX_t = cp.tile([128, 4, TQ], F32, tag="rzX")
            sqacc_t = cp.tile([128, TQ], F32, tag="sqacc")
            nc.vector.memset(eps_t[:], EPS)
            nc.vector.memset(shift_t[:], SHIFT)

            kT_tiles, qT_tiles, eb_tiles = {}, {}, {}
            m0_tiles, u_tiles, z4_tiles, at_tiles = {}, {}, {}, {}

            def emit_loads(g):
                """Load kT/qT/eb for the 4 heads of group g (3 DMAs)."""
                kT_t = kq.tile([D, 4, S], F16, tag="kT", name=f"kT_{g}")
                qT_t = kq.tile([D, 4, TQ], F16, tag="qT", name=f"qT_{g}")
                eb_t = ebp.tile([128, 4, 2, 4 * TQ], BF16, tag="eb",
                                name=f"eb_{g}")
                hs = slice(4 * g, 4 * g + 4)
                nc.sync.dma_start(out=kT_t[:],
                                  in_=kT_d[hs].rearrange("h d s -> d h s"))
                nc.sync.dma_start(out=qT_t[:],
                                  in_=qT_d[hs].rearrange("h d t -> d h t"))
                nc.sync.dma_start(out=eb_t[:],
                                  in_=eb_d[hs].rearrange("h p f x -> p h f x"))
                kT_tiles[g], qT_tiles[g], eb_tiles[g] = kT_t, qT_t, eb_t

            def emit_scores(h):
                g, h4 = h // 4, h % 4
                kT_t, qT_t, eb_t = kT_tiles[g], qT_tiles[g], eb_tiles[g]
                e_t = eu.tile([128, 2, 4 * TQ], BF16, tag="e", name=f"e_{h}")
                m0_t = eu.tile([128, 2, 4 * TQ], BF16, tag="m0",
                               name=f"m0_{h}")
                u_t = eu.tile([128, 2, 4 * TQ], BF16, tag="u", name=f"u_{h}")
                for hf in range(2):
                    w_ps = psw.tile([128, 4 * TQ], F32, tag="w",
                                    name=f"w_{h}_{hf}")
                    for sc4 in range(4):
                        sc = 4 * hf + sc4
                        nc.tensor.matmul(w_ps[:, sc4 * TQ:(sc4 + 1) * TQ],
                                         kT_t[:, h4, sc * 128:(sc + 1) * 128],
                                         qT_t[:, h4, :],
                                         start=True, stop=True,
                                         skip_group_check=True)
                    nc.scalar.activation(e_t[:, hf, :], w_ps[:], AF.Exp,
                                         bias=shift_t[:])
                    nc.vector.tensor_mul(m0_t[:, hf, :], e_t[:, hf, :],
                                         eb_t[:, h4, hf, :])
                    nc.vector.tensor_mul(u_t[:, hf, :], m0_t[:, hf, :],
                                         lawT_t[:, hf, :])
                m0_tiles[h], u_tiles[h] = m0_t, u_t

            def emit_post_pe(h):
                g, h4 = h // 4, h % 4
                m0_t, u_t = m0_tiles[h], u_tiles[h]
                if h4 == 0:
                    z4_tiles[g] = psz.tile([4, TQ], F32, tag="z",
                                           name=f"z4_{g}")
                z4_ps = z4_tiles[g]
                # one-hot lhsT: head h4 writes row h4, zeros elsewhere,
                # so the whole group accumulates into one [4, 256] tile
                for sc in range(8):
                    nc.tensor.matmul(z4_ps[:], E4_t[:, h4, :],
                                     m0_t[:, sc // 4,
                                          (sc % 4) * TQ:(sc % 4 + 1) * TQ],
                                     start=(h4 == 0 and sc == 0),
                                     stop=(h4 == 3 and sc == 7),
                                     skip_group_check=True)
                at_ps = psa.tile([D, TQ], F32, tag="at", name=f"at_{h}")
                for sc in range(8):
                    nc.tensor.matmul(at_ps[:], vS_t[:, sc, h, :],
                                     u_t[:, sc // 4,
                                         (sc % 4) * TQ:(sc % 4 + 1) * TQ],
                                     start=(sc == 0), stop=(sc == 7))
                at_tiles[h] = at_ps

            def emit_post_rest(h):
                del m0_tiles[h], u_tiles[h]
                at_ps = at_tiles.pop(h)
                at_sb = wp.tile([D, TQ], BF16, tag="atsb", name=f"atsb_{h}")
                nc.scalar.activation(at_sb[:], at_ps[:], AF.Copy)
                # stash into X[(h%4)*32+j, p, h//4, t] for out_proj lhsT
                for p, eng in ((0, nc.sync), (1, nc.sync), (2, nc.gpsimd)):
                    eng.dma_start(
                        out=X_t[(h % 4) * 32:(h % 4 + 1) * 32, p, h // 4, :],
                        in_=at_sb[p * 32:(p + 1) * 32, :])

            def emit_group_norm(g):
                # rz for the 4 heads of group g, in one reciprocal
                rz4_t = wp.tile([4, TQ], F32, tag="rz4", name=f"rz4_{g}")
                nc.vector.reciprocal(rz4_t[:], z4_tiles.pop(g)[:])
                # replicate each rz4 row 32x across partitions via one
                # stride-0-source DMA: rzX[hm*32+j, g, :] = rz4[hm, :]
                nc.gpsimd.dma_start(
                    out=rzX_t[:, g, :],
                    in_=rz4_t[:].unsqueeze(1).broadcast_to([4, 32, TQ]))
                # normalize X in place, then sumsq on normalized values
                for p in range(P):
                    nc.vector.tensor_mul(X_t[:, p, g, :], X_t[:, p, g, :],
                                         rzX_t[:, g, :])
                    if g == 0 and p == 0:
                        nc.vector.tensor_mul(sqacc_t[:], X_t[:, p, g, :],
                                             X_t[:, p, g, :])
                    else:
                        sq_t = wp.tile([128, TQ], BF16, tag="sq")
                        nc.vector.tensor_mul(sq_t[:], X_t[:, p, g, :],
                                             X_t[:, p, g, :])
                        nc.vector.tensor_add(sqacc_t[:], sqacc_t[:], sq_t[:])

            # ---- main loop, software-pipelined by one head ----
            emit_loads(0)
            nc.scalar.dma_start(out=lawT_t[:], in_=lawT_d)
            nc.scalar.dma_start(out=ones128_t[:], in_=ones128_d)
            nc.scalar.dma_start(out=E4_t[:], in_=E4_d)
            nc.scalar.dma_start(out=ones128f_t[:], in_=ones128f_d)
            emit_loads(1)
            for half in range(2):
                nc.scalar.dma_start(out=vS_t[:, 4 * half:4 * half + 4, :, :],
                                    in_=vS_d[:, 4 * half:4 * half + 4, :, :])
            nc.scalar.dma_start(out=WT_t[:], in_=WT_d)

            for h in range(H):
                if h % 4 == 0 and h // 4 + 2 < 4:
                    emit_loads(h // 4 + 2)
                if h >= 1:
                    emit_post_pe(h - 1)
                    emit_post_rest(h - 1)
                if h % 4 == 1 and h >= 5:
                    emit_group_norm(h // 4 - 1)
                emit_scores(h)
            emit_post_pe(H - 1)
            emit_post_rest(H - 1)
            emit_group_norm(3)

            # ---- inv = 1/sqrt(mean+eps), out_proj, scale, store ----
            ss_ps = [psz.tile([128, 1], F32, tag="z", name=f"ss{tb}")
                     for tb in range(2)]
            for tb in range(2):
                nc.tensor.matmul(ss_ps[tb][:],
                                 sqacc_t[:, tb * 128:(tb + 1) * 128],
                                 ones128f_t[:], start=True, stop=True)
            inv_t = []
            for tb in range(2):
                tmp_t = wp.tile([128, 1], F32, tag=f"tmp{tb}")
                nc.scalar.activation(tmp_t[:], ss_ps[tb][:], AF.Sqrt,
                                     scale=1.0 / HID, bias=eps_t[:])
                iv = wp.tile([128, 1], F32, tag=f"inv{tb}")
                nc.vector.reciprocal(iv[:], tmp_t[:])
                inv_t.append(iv)

            for tb in range(2):
                o_sb = wp.tile([128, P, HID], BF16, tag="osb",
                               name=f"osb_{tb}")
                for p in range(P):
                    o_ps = psw.tile([128, HID], F32, tag="w",
                                    name=f"o_{p}_{tb}")
                    for ci in range(4):
                        nc.tensor.matmul(o_ps[:],
                                         X_t[:, p, ci, tb * 128:(tb + 1) * 128],
                                         WT_t[:, ci, :],
                                         start=(ci == 0), stop=(ci == 3))
                    nc.vector.tensor_scalar_mul(o_sb[:, p, :], o_ps[:],
                                                inv_t[tb][:])
                nc.sync.dma_start(
                    out=out_d[tb * 128:(tb + 1) * 128, :, :], in_=o_sb[:])

    nc.compile()
    return nc


def _get_program():
    if "nc" not in _prog_cache:
        _prog_cache["nc"] = _build_program()
    return _prog_cache["nc"]


def _prepare_in_maps(q, k, v, attn_bias, key_padding_mask, outcell_index,
                     local_attention_weight, expand_mask, out_proj_weight,
                     attn_ln_weight):
    q = np.asarray(q, dtype=np.float32)
    k = np.asarray(k, dtype=np.float32)
    v = np.asarray(v, dtype=np.float32)
    attn_bias = np.asarray(attn_bias, dtype=np.float32)
    kpm = np.asarray(key_padding_mask)
    idx = np.asarray(outcell_index).astype(np.int64)
    law = np.asarray(local_attention_weight, dtype=np.float32)
    emask = np.asarray(expand_mask)
    W = np.asarray(out_proj_weight, dtype=np.float32)
    lnw = np.asarray(attn_ln_weight, dtype=np.float32)

    WT = np.ascontiguousarray((W * lnw[None, :]).T)  # [hid, o], ln folded
    ones128_np = np.ones((128, 1), dtype=ml_dtypes.bfloat16)
    E4_np = np.zeros((128, 4, 4), dtype=ml_dtypes.bfloat16)
    for i in range(4):
        E4_np[:, i, i] = 1
    ones128f_np = np.ones((128, 1), dtype=np.float32)

    in_maps = []
    for c in range(8):
        b, th = c // 2, c % 2
        tsl = slice(th * TQ, (th + 1) * TQ)

        # kT [H, 96, S]: kf[s, p, h*32+hd] with s-expansion host-gathered
        kf = np.concatenate([k[b], k[b][idx[b]]], axis=0)  # [S, P, HID]
        kT = kf.reshape(S, P, H, HD).transpose(2, 1, 3, 0).reshape(H, D, S)
        qT = q[b, tsl].reshape(TQ, P, H, HD).transpose(2, 1, 3, 0) \
            .reshape(H, D, TQ)

        # vS [128, 8, H, 96]: vS[part, sc, h, (p,hd)] = vf[sc*128+part, ...]
        vf = np.concatenate([v[b], v[b][idx[b]]], axis=0)  # [S, P, HID]
        vS = vf.reshape(8, 128, P, H, HD).transpose(1, 0, 3, 2, 4) \
            .reshape(128, 8, H, D)

        # masked bias [H, 256, S]
        bias_c = np.ascontiguousarray(attn_bias[b, :, tsl, :])
        kpmS = np.concatenate([kpm[b], emask[b]])           # [S]
        if kpmS.any():
            bias_c[:, :, kpmS] = NEG
        cut = law[b, tsl] <= CUTOFF                         # [256, S]
        if cut.any():
            bias_c[:, cut] = NEG
        # exp, transpose to [H, S, 256] -> [H, 128, 2, 1024]
        ebT = np.exp(bias_c.transpose(0, 2, 1)).reshape(H, 8, 128, TQ) \
            .transpose(0, 2, 1, 3).reshape(H, 128, 2, 4 * TQ)

        lawT = law[b, tsl].T.reshape(8, 128, TQ).transpose(1, 0, 2) \
            .reshape(128, 2, 4 * TQ)

        in_maps.append(dict(
            qT=qT.astype(np.float16),
            kT=np.ascontiguousarray(kT).astype(np.float16),
            vS=np.ascontiguousarray(vS).astype(ml_dtypes.bfloat16),
            eb=np.ascontiguousarray(ebT).astype(ml_dtypes.bfloat16),
            lawT=np.ascontiguousarray(lawT).astype(ml_dtypes.bfloat16),
            WT=WT.reshape(4, 128, HID).transpose(1, 0, 2).astype(
                ml_dtypes.bfloat16).copy(),
            ones128=ones128_np,
            E4=E4_np,
            ones128f=ones128f_np,
        ))
    return in_maps


def kernel(**inputs):
    in_maps = _prepare_in_maps(**inputs)
    nc = _get_program()
    res = run_bass_kernel_spmd(nc, in_maps, list(range(8)))

    out = np.empty((B, T, P, HID), dtype=np.float32)
    for c in range(8):
        b, th = c // 2, c % 2
        out[b, th * TQ:(th + 1) * TQ] = res.results[c]["out"].astype(np.float32)
    return out


# revision 18
# speedup vs baseline: 1.1440x; 1.0411x over previous
"""MemEffEquivariantAttention TRN2 Bass kernel (transposed-scores flow, v3).

Sharding: 8 cores = 4 batches x 2 query-token halves (fully data-parallel,
no collectives).

Scores are computed TRANSPOSED (wT[s, t] = kT_chunk.T @ qT) so the
attention probabilities already have s on partitions and feed the attn
matmul directly -- no transpose of probabilities is ever materialized.
Z (softmax denominator, per (head, t)) is recovered with ones-vector
matmuls over the s-partitions.

v4: Z-normalization is applied LATE: the main loop stashes UNNORMALIZED
attn rows into X (plain ACT copy), Z rows are collected per group of 4
heads into an SBUF tile, one DVE reciprocal per group computes rz, gpsimd
broadcasts it into an rzX tile, and 3 DVE multiplies per group normalize
X in place (sumsq follows on the normalized X).  This removes the
per-head reciprocal (1.7us each) / ACT ln-exp chain (which thrashed
activation tables at 1.3us per reload) from the critical path entirely.

v3 changes driven by the previous trace (137us: PE busy 86us with
439 small matmuls; DVE 81us of which 28us was `reciprocal` on
single-partition [1,256] tiles; 57us of HWDGE dma_start issue cost
spread over SP/ACT/Pool):
  - all heads use host-side exp(bias) (eb): the bias identity matmuls
    are gone (-32 matmuls); bias add is a bf16 DVE multiply.
  - 1/Z is computed as exp(-ln(Z)) on the ACT engine (ln and exp live in
    the same activation table set -> no table reloads), broadcast across
    partitions on the otherwise-idle gpsimd engine.  No DVE reciprocal.
  - q/k/bias loads are batched per group of 4 heads (3 dma_starts per
    group instead of 12), v/law/WT/out merged; X-stash DMAs split
    across the sync and gpsimd rings.
  - exp uses a constant -40 bias (softmax shift, folded out exactly by
    Z) to keep e/m0 in comfortable bf16/f32 range.
  - q/k fp16, output bf16 (upcast on host).
"""
import sys
sys.path.insert(0, "/opt/trn_rl_repo")

import numpy as np
import ml_dtypes

import concourse.bacc as bacc
import concourse.tile as tile
from concourse import mybir
from concourse.bass_utils import run_bass_kernel_spmd

F32 = mybir.dt.float32
F16 = mybir.dt.float16
BF16 = mybir.dt.bfloat16
AF = mybir.ActivationFunctionType

B, T, P, HID = 4, 512, 3, 512
HD, H = 32, 16
EXP, S = 512, 1024
TQ = 256            # query tokens per core
EPS = 1e-3
CUTOFF = 1e-5
NEG = -1e30
D = P * HD          # 96, per-head feature dim
SHIFT = -40.0       # constant softmax shift, cancels exactly via Z

_prog_cache = {}


def _build_program():
    nc = bacc.Bacc("TRN2", target_bir_lowering=False, debug=False)

    qT_d = nc.dram_tensor("qT", [H, D, TQ], F16, kind="ExternalInput").ap()
    kT_d = nc.dram_tensor("kT", [H, D, S], F16, kind="ExternalInput").ap()
    vS_d = nc.dram_tensor("vS", [128, 8, H, D], BF16, kind="ExternalInput").ap()
    # eb = exp(masked bias)^T, [H, s(part,chunk), t]
    eb_d = nc.dram_tensor("eb", [H, 128, 2, 4 * TQ], BF16, kind="ExternalInput").ap()
    lawT_d = nc.dram_tensor("lawT", [128, 2, 4 * TQ], BF16, kind="ExternalInput").ap()
    WT_d = nc.dram_tensor("WT", [128, 4, HID], BF16, kind="ExternalInput").ap()
    ones128_d = nc.dram_tensor("ones128", [128, 1], BF16, kind="ExternalInput").ap()
    E4_d = nc.dram_tensor("E4", [128, 4, 4], BF16, kind="ExternalInput").ap()
    ones128f_d = nc.dram_tensor("ones128f", [128, 1], F32, kind="ExternalInput").ap()
    out_d = nc.dram_tensor("out", [TQ, P, HID], BF16, kind="ExternalOutput").ap()

    with tile.TileContext(nc) as tc:
        with tc.tile_pool(name="const", bufs=1) as cp, \
             tc.tile_pool(name="kq", bufs=2) as kq, \
             tc.tile_pool(name="ebp", bufs=2) as ebp, \
             tc.tile_pool(name="eu", bufs=2) as eu, \
             tc.tile_pool(name="work", bufs=3) as wp, \
             tc.tile_pool(name="psw", bufs=2, space="PSUM") as psw, \
             tc.tile_pool(name="psa", bufs=2, space="PSUM") as psa, \
             tc.tile_pool(name="psz", bufs=2, space="PSUM") as psz:

            # ---- constants ----
            vS_t = cp.tile([128, 8, H, D], BF16, tag="vS")
            lawT_t = cp.tile([128, 2, 4 * TQ], BF16, tag="lawT")
            WT_t = cp.tile([128, 4, HID], BF16, tag="WT")
            ones128_t = cp.tile([128, 1], BF16, tag="o128")
            E4_t = cp.tile([128, 4, 4], BF16, tag="E4")
            ones128f_t = cp.tile([128, 1], F32, tag="o128f")
            eps_t = cp.tile([128, 1], F32, tag="eps")
            shift_t = cp.tile([128, 1], F32, tag="shift")
            X_t = cp.tile([128, P, 4, TQ], BF16, tag="X")
            rzX_t = cp.tile([128, 4, TQ], F32, tag="rzX")
            sqacc_t = cp.tile([128, TQ], F32, tag="sqacc")
            nc.vector.memset(eps_t[:], EPS)
            nc.vector.memset(shift_t[:], SHIFT)

            kT_tiles, qT_tiles, eb_tiles = {}, {}, {}
            m0_tiles, u_tiles, z4_tiles, at_tiles = {}, {}, {}, {}

            def emit_loads(g):
                """Load kT/qT/eb for the 4 heads of group g (3 DMAs)."""
                kT_t = kq.tile([D, 4, S], F16, tag="kT", name=f"kT_{g}")
                qT_t = kq.tile([D, 4, TQ], F16, tag="qT", name=f"qT_{g}")
                eb_t = ebp.tile([128, 4, 2, 4 * TQ], BF16, tag="eb",
                                name=f"eb_{g}")
                hs = slice(4 * g, 4 * g + 4)
                nc.sync.dma_start(out=kT_t[:],
                                  in_=kT_d[hs].rearrange("h d s -> d h s"))
                nc.sync.dma_start(out=qT_t[:],
                                  in_=qT_d[hs].rearrange("h d t -> d h t"))
                nc.sync.dma_start(out=eb_t[:],
                                  in_=eb_d[hs].rearrange("h p f x -> p h f x"))
                kT_tiles[g], qT_tiles[g], eb_tiles[g] = kT_t, qT_t, eb_t

            def emit_scores(h):
                g, h4 = h // 4, h % 4
                kT_t, qT_t, eb_t = kT_tiles[g], qT_tiles[g], eb_tiles[g]
                e_t = eu.tile([128, 2, 4 * TQ], BF16, tag="e", name=f"e_{h}")
                m0_t = eu.tile([128, 2, 4 * TQ], BF16, tag="m0",
                               name=f"m0_{h}")
                u_t = eu.tile([128, 2, 4 * TQ], BF16, tag="u", name=f"u_{h}")
                for hf in range(2):
                    w_ps = psw.tile([128, 4 * TQ], F32, tag="w",
                                    name=f"w_{h}_{hf}")
                    for sc4 in range(4):
                        sc = 4 * hf + sc4
                        nc.tensor.matmul(w_ps[:, sc4 * TQ:(sc4 + 1) * TQ],
                                         kT_t[:, h4, sc * 128:(sc + 1) * 128],
                                         qT_t[:, h4, :],
                                         start=True, stop=True,
                                         skip_group_check=True)
                    nc.scalar.activation(e_t[:, hf, :], w_ps[:], AF.Exp,
                                         bias=shift_t[:])
                    nc.vector.tensor_mul(m0_t[:, hf, :], e_t[:, hf, :],
                                         eb_t[:, h4, hf, :])
                    nc.vector.tensor_mul(u_t[:, hf, :], m0_t[:, hf, :],
                                         lawT_t[:, hf, :])
                m0_tiles[h], u_tiles[h] = m0_t, u_t

            def emit_post_pe(h):
                g, h4 = h // 4, h % 4
                m0_t, u_t = m0_tiles[h], u_tiles[h]
                if h4 == 0:
                    z4_tiles[g] = psz.tile([4, TQ], F32, tag="z",
                                           name=f"z4_{g}")
                z4_ps = z4_tiles[g]
                # one-hot lhsT: head h4 writes row h4, zeros elsewhere,
                # so the whole group accumulates into one [4, 256] tile
                for sc in range(8):
                    nc.tensor.matmul(z4_ps[:], E4_t[:, h4, :],
                                     m0_t[:, sc // 4,
                                          (sc % 4) * TQ:(sc % 4 + 1) * TQ],
                                     start=(h4 == 0 and sc == 0),
                                     stop=(h4 == 3 and sc == 7),
                                     skip_group_check=True)
                at_ps = psa.tile([D, TQ], F32, tag="at", name=f"at_{h}")
                for sc in range(8):
                    nc.tensor.matmul(at_ps[:], vS_t[:, sc, h, :],
                                     u_t[:, sc // 4,
                                         (sc % 4) * TQ:(sc % 4 + 1) * TQ],
                                     start=(sc == 0), stop=(sc == 7))
                at_tiles[h] = at_ps

            def emit_post_rest(h):
                del m0_tiles[h], u_tiles[h]
                at_ps = at_tiles.pop(h)
                at_sb = wp.tile([D, TQ], BF16, tag="atsb", name=f"atsb_{h}")
                nc.scalar.activation(at_sb[:], at_ps[:], AF.Copy)
                # stash into X[(h%4)*32+j, p, h//4, t] for out_proj lhsT
                for p, eng in ((0, nc.sync), (1, nc.sync), (2, nc.gpsimd)):
                    eng.dma_start(
                        out=X_t[(h % 4) * 32:(h % 4 + 1) * 32, p, h // 4, :],
                        in_=at_sb[p * 32:(p + 1) * 32, :])

            def emit_group_norm(g):
                # rz for the 4 heads of group g, in one reciprocal
                rz4_t = wp.tile([4, TQ], F32, tag="rz4", name=f"rz4_{g}")
                nc.vector.reciprocal(rz4_t[:], z4_tiles.pop(g)[:])
                # replicate each rz4 row 32x across partitions via one
                # stride-0-source DMA: rzX[hm*32+j, g, :] = rz4[hm, :]
                nc.gpsimd.dma_start(
                    out=rzX_t[:, g, :],
                    in_=rz4_t[:].unsqueeze(1).broadcast_to([4, 32, TQ]))
                # normalize X in place, then sumsq on normalized values
                for p in range(P):
                    nc.vector.tensor_mul(X_t[:, p, g, :], X_t[:, p, g, :],
                                         rzX_t[:, g, :])
                    if g == 0 and p == 0:
                        nc.vector.tensor_mul(sqacc_t[:], X_t[:, p, g, :],
                                             X_t[:, p, g, :])
                    else:
                        sq_t = wp.tile([128, TQ], BF16, tag="sq")
                        nc.vector.tensor_mul(sq_t[:], X_t[:, p, g, :],
                                             X_t[:, p, g, :])
                        nc.vector.tensor_add(sqacc_t[:], sqacc_t[:], sq_t[:])

            # ---- main loop, software-pipelined by one head ----
            emit_loads(0)
            nc.scalar.dma_start(out=lawT_t[:], in_=lawT_d)
            nc.scalar.dma_start(out=ones128_t[:], in_=ones128_d)
            nc.scalar.dma_start(out=E4_t[:], in_=E4_d)
            nc.scalar.dma_start(out=ones128f_t[:], in_=ones128f_d)
            emit_loads(1)
            for half in range(2):
                nc.scalar.dma_start(out=vS_t[:, 4 * half:4 * half + 4, :, :],
                                    in_=vS_d[:, 4 * half:4 * half + 4, :, :])
            nc.scalar.dma_start(out=WT_t[:], in_=WT_d)

            for h in range(H):
                if h % 4 == 0 and h // 4 + 2 < 4:
                    emit_loads(h // 4 + 2)
                emit_scores(h)
                if h >= 1:
                    emit_post_pe(h - 1)
                    emit_post_rest(h - 1)
                if h % 4 == 1 and h >= 5:
                    emit_group_norm(h // 4 - 1)
            emit_post_pe(H - 1)
            emit_post_rest(H - 1)
            emit_group_norm(3)

            # ---- inv = 1/sqrt(mean+eps), out_proj, scale, store ----
            ss_ps = [psz.tile([128, 1], F32, tag="z", name=f"ss{tb}")
                     for tb in range(2)]
            for tb in range(2):
                nc.tensor.matmul(ss_ps[tb][:],
                                 sqacc_t[:, tb * 128:(tb + 1) * 128],
                                 ones128f_t[:], start=True, stop=True)
            inv_t = []
            for tb in range(2):
                tmp_t = wp.tile([128, 1], F32, tag=f"tmp{tb}")
                nc.scalar.activation(tmp_t[:], ss_ps[tb][:], AF.Sqrt,
                                     scale=1.0 / HID, bias=eps_t[:])
                iv = wp.tile([128, 1], F32, tag=f"inv{tb}")
                nc.vector.reciprocal(iv[:], tmp_t[:])
                inv_t.append(iv)

            for tb in range(2):
                o_sb = wp.tile([128, P, HID], BF16, tag="osb",
                               name=f"osb_{tb}")
                for p in range(P):
                    o_ps = psw.tile([128, HID], F32, tag="w",
                                    name=f"o_{p}_{tb}")
                    for ci in range(4):
                        nc.tensor.matmul(o_ps[:],
                                         X_t[:, p, ci, tb * 128:(tb + 1) * 128],
                                         WT_t[:, ci, :],
                                         start=(ci == 0), stop=(ci == 3))
                    nc.vector.tensor_scalar_mul(o_sb[:, p, :], o_ps[:],
                                                inv_t[tb][:])
                nc.sync.dma_start(
                    out=out_d[tb * 128:(tb + 1) * 128, :, :], in_=o_sb[:])

    nc.compile()
    return nc


def _get_program():
    if "nc" not in _prog_cache:
        _prog_cache["nc"] = _build_program()
    return _prog_cache["nc"]


def _prepare_in_maps(q, k, v, attn_bias, key_padding_mask, outcell_index,
                     local_attention_weight, expand_mask, out_proj_weight,
                     attn_ln_weight):
    q = np.asarray(q, dtype=np.float32)
    k = np.asarray(k, dtype=np.float32)
    v = np.asarray(v, dtype=np.float32)
    attn_bias = np.asarray(attn_bias, dtype=np.float32)
    kpm = np.asarray(key_padding_mask)
    idx = np.asarray(outcell_index).astype(np.int64)
    law = np.asarray(local_attention_weight, dtype=np.float32)
    emask = np.asarray(expand_mask)
    W = np.asarray(out_proj_weight, dtype=np.float32)
    lnw = np.asarray(attn_ln_weight, dtype=np.float32)

    WT = np.ascontiguousarray((W * lnw[None, :]).T)  # [hid, o], ln folded
    ones128_np = np.ones((128, 1), dtype=ml_dtypes.bfloat16)
    E4_np = np.zeros((128, 4, 4), dtype=ml_dtypes.bfloat16)
    for i in range(4):
        E4_np[:, i, i] = 1
    ones128f_np = np.ones((128, 1), dtype=np.float32)

    in_maps = []
    for c in range(8):
        b, th = c // 2, c % 2
        tsl = slice(th * TQ, (th + 1) * TQ)

        # kT [H, 96, S]: kf[s, p, h*32+hd] with s-expansion host-gathered
        kf = np.concatenate([k[b], k[b][idx[b]]], axis=0)  # [S, P, HID]
        kT = kf.reshape(S, P, H, HD).transpose(2, 1, 3, 0).reshape(H, D, S)
        qT = q[b, tsl].reshape(TQ, P, H, HD).transpose(2, 1, 3, 0) \
            .reshape(H, D, TQ)

        # vS [128, 8, H, 96]: vS[part, sc, h, (p,hd)] = vf[sc*128+part, ...]
        vf = np.concatenate([v[b], v[b][idx[b]]], axis=0)  # [S, P, HID]
        vS = vf.reshape(8, 128, P, H, HD).transpose(1, 0, 3, 2, 4) \
            .reshape(128, 8, H, D)

        # masked bias [H, 256, S]
        bias_c = np.ascontiguousarray(attn_bias[b, :, tsl, :])
        kpmS = np.concatenate([kpm[b], emask[b]])           # [S]
        if kpmS.any():
            bias_c[:, :, kpmS] = NEG
        cut = law[b, tsl] <= CUTOFF                         # [256, S]
        if cut.any():
            bias_c[:, cut] = NEG
        # exp, transpose to [H, S, 256] -> [H, 128, 2, 1024]
        ebT = np.exp(bias_c.transpose(0, 2, 1)).reshape(H, 8, 128, TQ) \
            .transpose(0, 2, 1, 3).reshape(H, 128, 2, 4 * TQ)

        lawT = law[b, tsl].T.reshape(8, 128, TQ).transpose(1, 0, 2) \
            .reshape(128, 2, 4 * TQ)

        in_maps.append(dict(
            qT=qT.astype(np.float16),
            kT=np.ascontiguousarray(kT).astype(np.float16),
            vS=np.ascontiguousarray(vS).astype(ml_dtypes.bfloat16),
            eb=np.ascontiguousarray(ebT).astype(ml_dtypes.bfloat16),
            lawT=np.ascontiguousarray(lawT).astype(ml_dtypes.bfloat16),
            WT=WT.reshape(4, 128, HID).transpose(1, 0, 2).astype(
                ml_dtypes.bfloat16).copy(),
            ones128=ones128_np,
            E4=E4_np,
            ones128f=ones128f_np,
        ))
    return in_maps


def kernel(**inputs):
    in_maps = _prepare_in_maps(**inputs)
    nc = _get_program()
    res = run_bass_kernel_spmd(nc, in_maps, list(range(8)))

    out = np.empty((B, T, P, HID), dtype=np.float32)
    for c in range(8):
        b, th = c // 2, c % 2
        out[b, th * TQ:(th + 1) * TQ] = res.results[c]["out"].astype(np.float32)
    return out


# revision 19
# speedup vs baseline: 1.1627x; 1.0164x over previous
"""MemEffEquivariantAttention TRN2 Bass kernel (transposed-scores flow, v3).

Sharding: 8 cores = 4 batches x 2 query-token halves (fully data-parallel,
no collectives).

Scores are computed TRANSPOSED (wT[s, t] = kT_chunk.T @ qT) so the
attention probabilities already have s on partitions and feed the attn
matmul directly -- no transpose of probabilities is ever materialized.
Z (softmax denominator, per (head, t)) is recovered with ones-vector
matmuls over the s-partitions.

v4: Z-normalization is applied LATE: the main loop stashes UNNORMALIZED
attn rows into X (plain ACT copy), Z rows are collected per group of 4
heads into an SBUF tile, one DVE reciprocal per group computes rz, gpsimd
broadcasts it into an rzX tile, and 3 DVE multiplies per group normalize
X in place (sumsq follows on the normalized X).  This removes the
per-head reciprocal (1.7us each) / ACT ln-exp chain (which thrashed
activation tables at 1.3us per reload) from the critical path entirely.

v3 changes driven by the previous trace (137us: PE busy 86us with
439 small matmuls; DVE 81us of which 28us was `reciprocal` on
single-partition [1,256] tiles; 57us of HWDGE dma_start issue cost
spread over SP/ACT/Pool):
  - all heads use host-side exp(bias) (eb): the bias identity matmuls
    are gone (-32 matmuls); bias add is a bf16 DVE multiply.
  - 1/Z is computed as exp(-ln(Z)) on the ACT engine (ln and exp live in
    the same activation table set -> no table reloads), broadcast across
    partitions on the otherwise-idle gpsimd engine.  No DVE reciprocal.
  - q/k/bias loads are batched per group of 4 heads (3 dma_starts per
    group instead of 12), v/law/WT/out merged; X-stash DMAs split
    across the sync and gpsimd rings.
  - exp uses a constant -40 bias (softmax shift, folded out exactly by
    Z) to keep e/m0 in comfortable bf16/f32 range.
  - q/k fp16, output bf16 (upcast on host).
"""
import sys
sys.path.insert(0, "/opt/trn_rl_repo")

import numpy as np
import ml_dtypes

import concourse.bacc as bacc
import concourse.tile as tile
from concourse import mybir
from concourse.bass_utils import run_bass_kernel_spmd

F32 = mybir.dt.float32
F16 = mybir.dt.float16
BF16 = mybir.dt.bfloat16
AF = mybir.ActivationFunctionType

B, T, P, HID = 4, 512, 3, 512
HD, H = 32, 16
EXP, S = 512, 1024
TQ = 256            # query tokens per core
EPS = 1e-3
CUTOFF = 1e-5
NEG = -1e30
D = P * HD          # 96, per-head feature dim
SHIFT = -40.0       # constant softmax shift, cancels exactly via Z

_prog_cache = {}


def _build_program():
    nc = bacc.Bacc("TRN2", target_bir_lowering=False, debug=False)

    qT_d = nc.dram_tensor("qT", [H, D, TQ], F16, kind="ExternalInput").ap()
    kT_d = nc.dram_tensor("kT", [H, D, S], F16, kind="ExternalInput").ap()
    vS_d = nc.dram_tensor("vS", [128, 8, H, D], BF16, kind="ExternalInput").ap()
    # eb = exp(masked bias)^T, [H, s(part,chunk), t]
    eb_d = nc.dram_tensor("eb", [H, 128, 2, 4 * TQ], BF16, kind="ExternalInput").ap()
    lawT_d = nc.dram_tensor("lawT", [128, 2, 4 * TQ], BF16, kind="ExternalInput").ap()
    WT_d = nc.dram_tensor("WT", [128, 4, HID], BF16, kind="ExternalInput").ap()
    ones128_d = nc.dram_tensor("ones128", [128, 1], BF16, kind="ExternalInput").ap()
    E4_d = nc.dram_tensor("E4", [128, 4, 4], BF16, kind="ExternalInput").ap()
    ones128f_d = nc.dram_tensor("ones128f", [128, 1], F32, kind="ExternalInput").ap()
    out_d = nc.dram_tensor("out", [TQ, P, HID], BF16, kind="ExternalOutput").ap()

    with tile.TileContext(nc) as tc:
        with tc.tile_pool(name="const", bufs=1) as cp, \
             tc.tile_pool(name="kq", bufs=3) as kq, \
             tc.tile_pool(name="ebp", bufs=3) as ebp, \
             tc.tile_pool(name="eu", bufs=2) as eu, \
             tc.tile_pool(name="work", bufs=3) as wp, \
             tc.tile_pool(name="psw", bufs=2, space="PSUM") as psw, \
             tc.tile_pool(name="psa", bufs=2, space="PSUM") as psa, \
             tc.tile_pool(name="psz", bufs=2, space="PSUM") as psz:

            # ---- constants ----
            vS_t = cp.tile([128, 8, H, D], BF16, tag="vS")
            lawT_t = cp.tile([128, 2, 4 * TQ], BF16, tag="lawT")
            WT_t = cp.tile([128, 4, HID], BF16, tag="WT")
            ones128_t = cp.tile([128, 1], BF16, tag="o128")
            E4_t = cp.tile([128, 4, 4], BF16, tag="E4")
            ones128f_t = cp.tile([128, 1], F32, tag="o128f")
            eps_t = cp.tile([128, 1], F32, tag="eps")
            shift_t = cp.tile([128, 1], F32, tag="shift")
            X_t = cp.tile([128, P, 4, TQ], BF16, tag="X")
            rzX_t = cp.tile([128, 4, TQ], F32, tag="rzX")
            sqacc_t = cp.tile([128, TQ], F32, tag="sqacc")
            nc.vector.memset(eps_t[:], EPS)
            nc.vector.memset(shift_t[:], SHIFT)

            kT_tiles, qT_tiles, eb_tiles = {}, {}, {}
            m0_tiles, u_tiles, z4_tiles, at_tiles = {}, {}, {}, {}

            def emit_loads(g):
                """Load kT/qT/eb for the 4 heads of group g."""
                kT_t = kq.tile([D, 4, S], F16, tag="kT", name=f"kT_{g}")
                qT_t = kq.tile([D, 4, TQ], F16, tag="qT", name=f"qT_{g}")
                eb_t = ebp.tile([128, 4, 2, 4 * TQ], BF16, tag="eb",
                                name=f"eb_{g}")
                hs = slice(4 * g, 4 * g + 4)
                nc.sync.dma_start(out=kT_t[:],
                                  in_=kT_d[hs].rearrange("h d s -> d h s"))
                nc.sync.dma_start(out=qT_t[:],
                                  in_=qT_d[hs].rearrange("h d t -> d h t"))
                if g == 0:
                    # per-head chunks so head 0 can start ~2.5us earlier
                    for h4 in range(4):
                        nc.sync.dma_start(out=eb_t[:, h4, :, :],
                                          in_=eb_d[h4])
                else:
                    nc.sync.dma_start(out=eb_t[:],
                                      in_=eb_d[hs].rearrange("h p f x -> p h f x"))
                kT_tiles[g], qT_tiles[g], eb_tiles[g] = kT_t, qT_t, eb_t

            def emit_scores(h):
                g, h4 = h // 4, h % 4
                kT_t, qT_t, eb_t = kT_tiles[g], qT_tiles[g], eb_tiles[g]
                e_t = eu.tile([128, 2, 4 * TQ], BF16, tag="e", name=f"e_{h}")
                m0_t = eu.tile([128, 2, 4 * TQ], BF16, tag="m0",
                               name=f"m0_{h}")
                u_t = eu.tile([128, 2, 4 * TQ], BF16, tag="u", name=f"u_{h}")
                for hf in range(2):
                    w_ps = psw.tile([128, 4 * TQ], F32, tag="w",
                                    name=f"w_{h}_{hf}")
                    for sc4 in range(4):
                        sc = 4 * hf + sc4
                        nc.tensor.matmul(w_ps[:, sc4 * TQ:(sc4 + 1) * TQ],
                                         kT_t[:, h4, sc * 128:(sc + 1) * 128],
                                         qT_t[:, h4, :],
                                         start=True, stop=True,
                                         skip_group_check=True)
                    nc.scalar.activation(e_t[:, hf, :], w_ps[:], AF.Exp,
                                         bias=shift_t[:])
                    nc.vector.tensor_mul(m0_t[:, hf, :], e_t[:, hf, :],
                                         eb_t[:, h4, hf, :])
                    nc.vector.tensor_mul(u_t[:, hf, :], m0_t[:, hf, :],
                                         lawT_t[:, hf, :])
                m0_tiles[h], u_tiles[h] = m0_t, u_t

            def emit_post_pe(h):
                g, h4 = h // 4, h % 4
                m0_t, u_t = m0_tiles[h], u_tiles[h]
                if h4 == 0:
                    z4_tiles[g] = psz.tile([4, TQ], F32, tag="z",
                                           name=f"z4_{g}")
                z4_ps = z4_tiles[g]
                # one-hot lhsT: head h4 writes row h4, zeros elsewhere,
                # so the whole group accumulates into one [4, 256] tile
                for sc in range(8):
                    nc.tensor.matmul(z4_ps[:], E4_t[:, h4, :],
                                     m0_t[:, sc // 4,
                                          (sc % 4) * TQ:(sc % 4 + 1) * TQ],
                                     start=(h4 == 0 and sc == 0),
                                     stop=(h4 == 3 and sc == 7),
                                     skip_group_check=True)
                at_ps = psa.tile([D, TQ], F32, tag="at", name=f"at_{h}")
                for sc in range(8):
                    nc.tensor.matmul(at_ps[:], vS_t[:, sc, h, :],
                                     u_t[:, sc // 4,
                                         (sc % 4) * TQ:(sc % 4 + 1) * TQ],
                                     start=(sc == 0), stop=(sc == 7))
                at_tiles[h] = at_ps

            def emit_post_rest(h):
                del m0_tiles[h], u_tiles[h]
                at_ps = at_tiles.pop(h)
                at_sb = wp.tile([D, TQ], BF16, tag="atsb", name=f"atsb_{h}")
                nc.scalar.activation(at_sb[:], at_ps[:], AF.Copy)
                # stash into X[(h%4)*32+j, p, h//4, t] for out_proj lhsT
                for p in range(P):
                    nc.sync.dma_start(
                        out=X_t[(h % 4) * 32:(h % 4 + 1) * 32, p, h // 4, :],
                        in_=at_sb[p * 32:(p + 1) * 32, :])

            def emit_group_norm(g):
                # rz for the 4 heads of group g, in one reciprocal
                rz4_t = wp.tile([4, TQ], F32, tag="rz4", name=f"rz4_{g}")
                nc.vector.reciprocal(rz4_t[:], z4_tiles.pop(g)[:])
                # replicate each rz4 row 32x across partitions via one
                # stride-0-source DMA: rzX[hm*32+j, g, :] = rz4[hm, :]
                nc.gpsimd.dma_start(
                    out=rzX_t[:, g, :],
                    in_=rz4_t[:].unsqueeze(1).broadcast_to([4, 32, TQ]))
                # normalize X in place, then sumsq on normalized values
                for p in range(P):
                    nc.vector.tensor_mul(X_t[:, p, g, :], X_t[:, p, g, :],
                                         rzX_t[:, g, :])
                    if g == 0 and p == 0:
                        nc.vector.tensor_mul(sqacc_t[:], X_t[:, p, g, :],
                                             X_t[:, p, g, :])
                    else:
                        sq_t = wp.tile([128, TQ], BF16, tag="sq")
                        nc.vector.tensor_mul(sq_t[:], X_t[:, p, g, :],
                                             X_t[:, p, g, :])
                        nc.vector.tensor_add(sqacc_t[:], sqacc_t[:], sq_t[:])

            # ---- main loop, software-pipelined by one head ----
            emit_loads(0)
            nc.scalar.dma_start(out=lawT_t[:], in_=lawT_d)
            nc.scalar.dma_start(out=ones128_t[:], in_=ones128_d)
            nc.scalar.dma_start(out=E4_t[:], in_=E4_d)
            nc.scalar.dma_start(out=ones128f_t[:], in_=ones128f_d)
            emit_loads(1)
            for half in range(2):
                nc.scalar.dma_start(out=vS_t[:, 4 * half:4 * half + 4, :, :],
                                    in_=vS_d[:, 4 * half:4 * half + 4, :, :])
            nc.scalar.dma_start(out=WT_t[:], in_=WT_d)

            for h in range(H):
                if h % 4 == 0 and h // 4 + 2 < 4:
                    emit_loads(h // 4 + 2)
                emit_scores(h)
                if h >= 1:
                    emit_post_pe(h - 1)
                    emit_post_rest(h - 1)
                if h % 4 == 1 and h >= 5:
                    emit_group_norm(h // 4 - 1)
            emit_post_pe(H - 1)
            emit_post_rest(H - 1)
            emit_group_norm(3)

            # ---- inv = 1/sqrt(mean+eps), out_proj, scale, store ----
            ss_ps = [psz.tile([128, 1], F32, tag="z", name=f"ss{tb}")
                     for tb in range(2)]
            for tb in range(2):
                nc.tensor.matmul(ss_ps[tb][:],
                                 sqacc_t[:, tb * 128:(tb + 1) * 128],
                                 ones128f_t[:], start=True, stop=True)
            inv_t = []
            for tb in range(2):
                tmp_t = wp.tile([128, 1], F32, tag=f"tmp{tb}")
                nc.scalar.activation(tmp_t[:], ss_ps[tb][:], AF.Sqrt,
                                     scale=1.0 / HID, bias=eps_t[:])
                iv = wp.tile([128, 1], F32, tag=f"inv{tb}")
                nc.vector.reciprocal(iv[:], tmp_t[:])
                inv_t.append(iv)

            for tb in range(2):
                o_sb = wp.tile([128, P, HID], BF16, tag="osb",
                               name=f"osb_{tb}")
                for p in range(P):
                    o_ps = psw.tile([128, HID], F32, tag="w",
                                    name=f"o_{p}_{tb}")
                    for ci in range(4):
                        nc.tensor.matmul(o_ps[:],
                                         X_t[:, p, ci, tb * 128:(tb + 1) * 128],
                                         WT_t[:, ci, :],
                                         start=(ci == 0), stop=(ci == 3))
                    nc.vector.tensor_scalar_mul(o_sb[:, p, :], o_ps[:],
                                                inv_t[tb][:])
                nc.sync.dma_start(
                    out=out_d[tb * 128:(tb + 1) * 128, :, :], in_=o_sb[:])

    nc.compile()
    return nc


def _get_program():
    if "nc" not in _prog_cache:
        _prog_cache["nc"] = _build_program()
    return _prog_cache["nc"]


def _prepare_in_maps(q, k, v, attn_bias, key_padding_mask, outcell_index,
                     local_attention_weight, expand_mask, out_proj_weight,
                     attn_ln_weight):
    q = np.asarray(q, dtype=np.float32)
    k = np.asarray(k, dtype=np.float32)
    v = np.asarray(v, dtype=np.float32)
    attn_bias = np.asarray(attn_bias, dtype=np.float32)
    kpm = np.asarray(key_padding_mask)
    idx = np.asarray(outcell_index).astype(np.int64)
    law = np.asarray(local_attention_weight, dtype=np.float32)
    emask = np.asarray(expand_mask)
    W = np.asarray(out_proj_weight, dtype=np.float32)
    lnw = np.asarray(attn_ln_weight, dtype=np.float32)

    WT = np.ascontiguousarray((W * lnw[None, :]).T)  # [hid, o], ln folded
    ones128_np = np.ones((128, 1), dtype=ml_dtypes.bfloat16)
    E4_np = np.zeros((128, 4, 4), dtype=ml_dtypes.bfloat16)
    for i in range(4):
        E4_np[:, i, i] = 1
    ones128f_np = np.ones((128, 1), dtype=np.float32)

    in_maps = []
    for c in range(8):
        b, th = c // 2, c % 2
        tsl = slice(th * TQ, (th + 1) * TQ)

        # kT [H, 96, S]: kf[s, p, h*32+hd] with s-expansion host-gathered
        kf = np.concatenate([k[b], k[b][idx[b]]], axis=0)  # [S, P, HID]
        kT = kf.reshape(S, P, H, HD).transpose(2, 1, 3, 0).reshape(H, D, S)
        qT = q[b, tsl].reshape(TQ, P, H, HD).transpose(2, 1, 3, 0) \
            .reshape(H, D, TQ)

        # vS [128, 8, H, 96]: vS[part, sc, h, (p,hd)] = vf[sc*128+part, ...]
        vf = np.concatenate([v[b], v[b][idx[b]]], axis=0)  # [S, P, HID]
        vS = vf.reshape(8, 128, P, H, HD).transpose(1, 0, 3, 2, 4) \
            .reshape(128, 8, H, D)

        # masked bias [H, 256, S]
        bias_c = np.ascontiguousarray(attn_bias[b, :, tsl, :])
        kpmS = np.concatenate([kpm[b], emask[b]])           # [S]
        if kpmS.any():
            bias_c[:, :, kpmS] = NEG
        cut = law[b, tsl] <= CUTOFF                         # [256, S]
        if cut.any():
            bias_c[:, cut] = NEG
        # exp, transpose to [H, S, 256] -> [H, 128, 2, 1024]
        ebT = np.exp(bias_c.transpose(0, 2, 1)).reshape(H, 8, 128, TQ) \
            .transpose(0, 2, 1, 3).reshape(H, 128, 2, 4 * TQ)

        lawT = law[b, tsl].T.reshape(8, 128, TQ).transpose(1, 0, 2) \
            .reshape(128, 2, 4 * TQ)

        in_maps.append(dict(
            qT=qT.astype(np.float16),
            kT=np.ascontiguousarray(kT).astype(np.float16),
            vS=np.ascontiguousarray(vS).astype(ml_dtypes.bfloat16),
            eb=np.ascontiguousarray(ebT).astype(ml_dtypes.bfloat16),
            lawT=np.ascontiguousarray(lawT).astype(ml_dtypes.bfloat16),
            WT=WT.reshape(4, 128, HID).transpose(1, 0, 2).astype(
                ml_dtypes.bfloat16).copy(),
            ones128=ones128_np,
            E4=E4_np,
            ones128f=ones128f_np,
        ))
    return in_maps


def kernel(**inputs):
    in_maps = _prepare_in_maps(**inputs)
    nc = _get_program()
    res = run_bass_kernel_spmd(nc, in_maps, list(range(8)))

    out = np.empty((B, T, P, HID), dtype=np.float32)
    for c in range(8):
        b, th = c // 2, c % 2
        out[b, th * TQ:(th + 1) * TQ] = res.results[c]["out"].astype(np.float32)
    return out


# revision 20
# speedup vs baseline: 1.1777x; 1.0129x over previous
"""MemEffEquivariantAttention TRN2 Bass kernel (transposed-scores flow, v3).

Sharding: 8 cores = 4 batches x 2 query-token halves (fully data-parallel,
no collectives).

Scores are computed TRANSPOSED (wT[s, t] = kT_chunk.T @ qT) so the
attention probabilities already have s on partitions and feed the attn
matmul directly -- no transpose of probabilities is ever materialized.
Z (softmax denominator, per (head, t)) is recovered with ones-vector
matmuls over the s-partitions.

v4: Z-normalization is applied LATE: the main loop stashes UNNORMALIZED
attn rows into X (plain ACT copy), Z rows are collected per group of 4
heads into an SBUF tile, one DVE reciprocal per group computes rz, gpsimd
broadcasts it into an rzX tile, and 3 DVE multiplies per group normalize
X in place (sumsq follows on the normalized X).  This removes the
per-head reciprocal (1.7us each) / ACT ln-exp chain (which thrashed
activation tables at 1.3us per reload) from the critical path entirely.

v3 changes driven by the previous trace (137us: PE busy 86us with
439 small matmuls; DVE 81us of which 28us was `reciprocal` on
single-partition [1,256] tiles; 57us of HWDGE dma_start issue cost
spread over SP/ACT/Pool):
  - all heads use host-side exp(bias) (eb): the bias identity matmuls
    are gone (-32 matmuls); bias add is a bf16 DVE multiply.
  - 1/Z is computed as exp(-ln(Z)) on the ACT engine (ln and exp live in
    the same activation table set -> no table reloads), broadcast across
    partitions on the otherwise-idle gpsimd engine.  No DVE reciprocal.
  - q/k/bias loads are batched per group of 4 heads (3 dma_starts per
    group instead of 12), v/law/WT/out merged; X-stash DMAs split
    across the sync and gpsimd rings.
  - exp uses a constant -40 bias (softmax shift, folded out exactly by
    Z) to keep e/m0 in comfortable bf16/f32 range.
  - q/k fp16, output bf16 (upcast on host).
"""
import sys
sys.path.insert(0, "/opt/trn_rl_repo")

import numpy as np
import ml_dtypes

import concourse.bacc as bacc
import concourse.tile as tile
from concourse import mybir
from concourse.bass_utils import run_bass_kernel_spmd

F32 = mybir.dt.float32
F16 = mybir.dt.float16
BF16 = mybir.dt.bfloat16
AF = mybir.ActivationFunctionType

B, T, P, HID = 4, 512, 3, 512
HD, H = 32, 16
EXP, S = 512, 1024
TQ = 256            # query tokens per core
EPS = 1e-3
CUTOFF = 1e-5
NEG = -1e30
D = P * HD          # 96, per-head feature dim
SHIFT = -40.0       # constant softmax shift, cancels exactly via Z

_prog_cache = {}


def _build_program():
    nc = bacc.Bacc("TRN2", target_bir_lowering=False, debug=False)

    qT_d = nc.dram_tensor("qT", [H, D, TQ], F16, kind="ExternalInput").ap()
    kT_d = nc.dram_tensor("kT", [H, D, S], F16, kind="ExternalInput").ap()
    vS_d = nc.dram_tensor("vS", [128, 8, H, D], BF16, kind="ExternalInput").ap()
    # eb = exp(masked bias)^T, [H, s(part,chunk), t]
    eb_d = nc.dram_tensor("eb", [H, 128, 2, 4 * TQ], BF16, kind="ExternalInput").ap()
    lawT_d = nc.dram_tensor("lawT", [128, 2, 4 * TQ], BF16, kind="ExternalInput").ap()
    WT_d = nc.dram_tensor("WT", [128, 4, HID], BF16, kind="ExternalInput").ap()
    ones128_d = nc.dram_tensor("ones128", [128, 1], BF16, kind="ExternalInput").ap()
    E4_d = nc.dram_tensor("E4", [128, 4, 4], BF16, kind="ExternalInput").ap()
    ones128f_d = nc.dram_tensor("ones128f", [128, 1], F32, kind="ExternalInput").ap()
    out_d = nc.dram_tensor("out", [TQ, P, HID], BF16, kind="ExternalOutput").ap()

    with tile.TileContext(nc) as tc:
        with tc.tile_pool(name="const", bufs=1) as cp, \
             tc.tile_pool(name="kq", bufs=3) as kq, \
             tc.tile_pool(name="ebp", bufs=3) as ebp, \
             tc.tile_pool(name="eu", bufs=2) as eu, \
             tc.tile_pool(name="work", bufs=3) as wp, \
             tc.tile_pool(name="psw", bufs=2, space="PSUM") as psw, \
             tc.tile_pool(name="psa", bufs=2, space="PSUM") as psa, \
             tc.tile_pool(name="psz", bufs=2, space="PSUM") as psz:

            # ---- constants ----
            vS_t = cp.tile([128, 8, H, D], BF16, tag="vS")
            lawT_t = cp.tile([128, 2, 4 * TQ], BF16, tag="lawT")
            WT_t = cp.tile([128, 4, HID], BF16, tag="WT")
            ones128_t = cp.tile([128, 1], BF16, tag="o128")
            E4_t = cp.tile([128, 4, 4], BF16, tag="E4")
            ones128f_t = cp.tile([128, 1], F32, tag="o128f")
            eps_t = cp.tile([128, 1], F32, tag="eps")
            shift_t = cp.tile([128, 1], F32, tag="shift")
            X_t = cp.tile([128, P, 4, TQ], BF16, tag="X")
            rzX_t = cp.tile([128, 4, TQ], F32, tag="rzX")
            sqacc_t = cp.tile([128, TQ], F32, tag="sqacc")
            nc.vector.memset(eps_t[:], EPS)
            nc.vector.memset(shift_t[:], SHIFT)

            kT_tiles, qT_tiles, eb_tiles = {}, {}, {}
            m0_tiles, u_tiles, z4_tiles, at_tiles = {}, {}, {}, {}

            def emit_loads(g):
                """Load kT/qT/eb for the 4 heads of group g."""
                kT_t = kq.tile([D, 4, S], F16, tag="kT", name=f"kT_{g}")
                qT_t = kq.tile([D, 4, TQ], F16, tag="qT", name=f"qT_{g}")
                eb_t = ebp.tile([128, 4, 2, 4 * TQ], BF16, tag="eb",
                                name=f"eb_{g}")
                hs = slice(4 * g, 4 * g + 4)
                nc.sync.dma_start(out=kT_t[:],
                                  in_=kT_d[hs].rearrange("h d s -> d h s"))
                nc.sync.dma_start(out=qT_t[:],
                                  in_=qT_d[hs].rearrange("h d t -> d h t"))
                if g == 0:
                    # per-head chunks so head 0 can start ~2.5us earlier
                    for h4 in range(4):
                        nc.sync.dma_start(out=eb_t[:, h4, :, :],
                                          in_=eb_d[h4])
                else:
                    nc.sync.dma_start(out=eb_t[:],
                                      in_=eb_d[hs].rearrange("h p f x -> p h f x"))
                kT_tiles[g], qT_tiles[g], eb_tiles[g] = kT_t, qT_t, eb_t

            def emit_scores(h):
                g, h4 = h // 4, h % 4
                kT_t, qT_t, eb_t = kT_tiles[g], qT_tiles[g], eb_tiles[g]
                e_t = eu.tile([128, 2, 4 * TQ], BF16, tag="e", name=f"e_{h}")
                m0_t = eu.tile([128, 2, 4 * TQ], BF16, tag="m0",
                               name=f"m0_{h}")
                u_t = eu.tile([128, 2, 4 * TQ], BF16, tag="u", name=f"u_{h}")
                for hf in range(2):
                    w_ps = psw.tile([128, 4 * TQ], F32, tag="w",
                                    name=f"w_{h}_{hf}")
                    for sc4 in range(4):
                        sc = 4 * hf + sc4
                        nc.tensor.matmul(w_ps[:, sc4 * TQ:(sc4 + 1) * TQ],
                                         kT_t[:, h4, sc * 128:(sc + 1) * 128],
                                         qT_t[:, h4, :],
                                         start=True, stop=True,
                                         skip_group_check=True)
                    nc.scalar.activation(e_t[:, hf, :], w_ps[:], AF.Exp,
                                         bias=shift_t[:])
                    nc.vector.tensor_mul(m0_t[:, hf, :], e_t[:, hf, :],
                                         eb_t[:, h4, hf, :])
                    nc.vector.tensor_mul(u_t[:, hf, :], m0_t[:, hf, :],
                                         lawT_t[:, hf, :])
                m0_tiles[h], u_tiles[h] = m0_t, u_t

            def emit_post_pe(h):
                g, h4 = h // 4, h % 4
                m0_t, u_t = m0_tiles[h], u_tiles[h]
                if h4 == 0:
                    z4_tiles[g] = psz.tile([4, TQ], F32, tag="z",
                                           name=f"z4_{g}")
                z4_ps = z4_tiles[g]
                # one-hot lhsT: head h4 writes row h4, zeros elsewhere,
                # so the whole group accumulates into one [4, 256] tile
                for sc in range(8):
                    nc.tensor.matmul(z4_ps[:], E4_t[:, h4, :],
                                     m0_t[:, sc // 4,
                                          (sc % 4) * TQ:(sc % 4 + 1) * TQ],
                                     start=(h4 == 0 and sc == 0),
                                     stop=(h4 == 3 and sc == 7),
                                     skip_group_check=True)
                at_ps = psa.tile([D, TQ], F32, tag="at", name=f"at_{h}")
                for sc in range(8):
                    nc.tensor.matmul(at_ps[:], vS_t[:, sc, h, :],
                                     u_t[:, sc // 4,
                                         (sc % 4) * TQ:(sc % 4 + 1) * TQ],
                                     start=(sc == 0), stop=(sc == 7))
                at_tiles[h] = at_ps

            def emit_post_rest(h):
                del m0_tiles[h], u_tiles[h]
                at_ps = at_tiles.pop(h)
                at_sb = wp.tile([D, TQ], BF16, tag="atsb", name=f"atsb_{h}")
                nc.scalar.activation(at_sb[:], at_ps[:], AF.Copy)
                # stash into X[(h%4)*32+j, p, h//4, t] for out_proj lhsT
                for p in range(P):
                    nc.sync.dma_start(
                        out=X_t[(h % 4) * 32:(h % 4 + 1) * 32, p, h // 4, :],
                        in_=at_sb[p * 32:(p + 1) * 32, :])

            def emit_rz(g):
                # rz for the 4 heads of group g, in one reciprocal, then
                # replicate each rz4 row 32x across partitions via one
                # stride-0-source DMA: rzX[hm*32+j, g, :] = rz4[hm, :]
                rz4_t = wp.tile([4, TQ], F32, tag="rz4", name=f"rz4_{g}")
                nc.vector.reciprocal(rz4_t[:], z4_tiles.pop(g)[:])
                nc.sync.dma_start(
                    out=rzX_t[:, g, :],
                    in_=rz4_t[:].unsqueeze(1).broadcast_to([4, 32, TQ]))

            def emit_xnorm(g):
                # normalize X in place, then sumsq on normalized values
                # (one group behind emit_rz so the rzX DMA is long done)
                for p in range(P):
                    nc.vector.tensor_mul(X_t[:, p, g, :], X_t[:, p, g, :],
                                         rzX_t[:, g, :])
                    if g == 0 and p == 0:
                        nc.vector.tensor_mul(sqacc_t[:], X_t[:, p, g, :],
                                             X_t[:, p, g, :])
                    else:
                        sq_t = wp.tile([128, TQ], BF16, tag="sq")
                        nc.vector.tensor_mul(sq_t[:], X_t[:, p, g, :],
                                             X_t[:, p, g, :])
                        nc.vector.tensor_add(sqacc_t[:], sqacc_t[:], sq_t[:])

            # ---- main loop, software-pipelined by one head ----
            emit_loads(0)
            nc.scalar.dma_start(out=lawT_t[:], in_=lawT_d)
            nc.scalar.dma_start(out=ones128_t[:], in_=ones128_d)
            nc.scalar.dma_start(out=E4_t[:], in_=E4_d)
            nc.scalar.dma_start(out=ones128f_t[:], in_=ones128f_d)
            emit_loads(1)
            for half in range(2):
                nc.scalar.dma_start(out=vS_t[:, 4 * half:4 * half + 4, :, :],
                                    in_=vS_d[:, 4 * half:4 * half + 4, :, :])
            nc.scalar.dma_start(out=WT_t[:], in_=WT_d)

            for h in range(H):
                if h % 4 == 0 and h // 4 + 2 < 4:
                    emit_loads(h // 4 + 2)
                emit_scores(h)
                if h >= 1:
                    emit_post_pe(h - 1)
                    emit_post_rest(h - 1)
                if h % 4 == 1 and h >= 5:
                    emit_rz(h // 4 - 1)
                    if h >= 9:
                        emit_xnorm(h // 4 - 2)
            emit_post_pe(H - 1)
            emit_post_rest(H - 1)
            emit_rz(3)
            emit_xnorm(2)
            emit_xnorm(3)

            # ---- inv = 1/sqrt(mean+eps), out_proj, scale, store ----
            ss_ps = [psz.tile([128, 1], F32, tag="z", name=f"ss{tb}")
                     for tb in range(2)]
            for tb in range(2):
                nc.tensor.matmul(ss_ps[tb][:],
                                 sqacc_t[:, tb * 128:(tb + 1) * 128],
                                 ones128f_t[:], start=True, stop=True)
            inv_t = []
            for tb in range(2):
                tmp_t = wp.tile([128, 1], F32, tag=f"tmp{tb}")
                nc.scalar.activation(tmp_t[:], ss_ps[tb][:], AF.Sqrt,
                                     scale=1.0 / HID, bias=eps_t[:])
                iv = wp.tile([128, 1], F32, tag=f"inv{tb}")
                nc.vector.reciprocal(iv[:], tmp_t[:])
                inv_t.append(iv)

            for tb in range(2):
                o_sb = wp.tile([128, P, HID], BF16, tag="osb",
                               name=f"osb_{tb}")
                for p in range(P):
                    o_ps = psw.tile([128, HID], F32, tag="w",
                                    name=f"o_{p}_{tb}")
                    for ci in range(4):
                        nc.tensor.matmul(o_ps[:],
                                         X_t[:, p, ci, tb * 128:(tb + 1) * 128],
                                         WT_t[:, ci, :],
                                         start=(ci == 0), stop=(ci == 3))
                    nc.vector.tensor_scalar_mul(o_sb[:, p, :], o_ps[:],
                                                inv_t[tb][:])
                nc.sync.dma_start(
                    out=out_d[tb * 128:(tb + 1) * 128, :, :], in_=o_sb[:])

    nc.compile()
    return nc


def _get_program():
    if "nc" not in _prog_cache:
        _prog_cache["nc"] = _build_program()
    return _prog_cache["nc"]


def _prepare_in_maps(q, k, v, attn_bias, key_padding_mask, outcell_index,
                     local_attention_weight, expand_mask, out_proj_weight,
                     attn_ln_weight):
    q = np.asarray(q, dtype=np.float32)
    k = np.asarray(k, dtype=np.float32)
    v = np.asarray(v, dtype=np.float32)
    attn_bias = np.asarray(attn_bias, dtype=np.float32)
    kpm = np.asarray(key_padding_mask)
    idx = np.asarray(outcell_index).astype(np.int64)
    law = np.asarray(local_attention_weight, dtype=np.float32)
    emask = np.asarray(expand_mask)
    W = np.asarray(out_proj_weight, dtype=np.float32)
    lnw = np.asarray(attn_ln_weight, dtype=np.float32)

    WT = np.ascontiguousarray((W * lnw[None, :]).T)  # [hid, o], ln folded
    ones128_np = np.ones((128, 1), dtype=ml_dtypes.bfloat16)
    E4_np = np.zeros((128, 4, 4), dtype=ml_dtypes.bfloat16)
    for i in range(4):
        E4_np[:, i, i] = 1
    ones128f_np = np.ones((128, 1), dtype=np.float32)

    in_maps = []
    for c in range(8):
        b, th = c // 2, c % 2
        tsl = slice(th * TQ, (th + 1) * TQ)

        # kT [H, 96, S]: kf[s, p, h*32+hd] with s-expansion host-gathered
        kf = np.concatenate([k[b], k[b][idx[b]]], axis=0)  # [S, P, HID]
        kT = kf.reshape(S, P, H, HD).transpose(2, 1, 3, 0).reshape(H, D, S)
        qT = q[b, tsl].reshape(TQ, P, H, HD).transpose(2, 1, 3, 0) \
            .reshape(H, D, TQ)

        # vS [128, 8, H, 96]: vS[part, sc, h, (p,hd)] = vf[sc*128+part, ...]
        vf = np.concatenate([v[b], v[b][idx[b]]], axis=0)  # [S, P, HID]
        vS = vf.reshape(8, 128, P, H, HD).transpose(1, 0, 3, 2, 4) \
            .reshape(128, 8, H, D)

        # masked bias [H, 256, S]
        bias_c = np.ascontiguousarray(attn_bias[b, :, tsl, :])
        kpmS = np.concatenate([kpm[b], emask[b]])           # [S]
        if kpmS.any():
            bias_c[:, :, kpmS] = NEG
        cut = law[b, tsl] <= CUTOFF                         # [256, S]
        if cut.any():
            bias_c[:, cut] = NEG
        # exp, transpose to [H, S, 256] -> [H, 128, 2, 1024]
        ebT = np.exp(bias_c.transpose(0, 2, 1)).reshape(H, 8, 128, TQ) \
            .transpose(0, 2, 1, 3).reshape(H, 128, 2, 4 * TQ)

        lawT = law[b, tsl].T.reshape(8, 128, TQ).transpose(1, 0, 2) \
            .reshape(128, 2, 4 * TQ)

        in_maps.append(dict(
            qT=qT.astype(np.float16),
            kT=np.ascontiguousarray(kT).astype(np.float16),
            vS=np.ascontiguousarray(vS).astype(ml_dtypes.bfloat16),
            eb=np.ascontiguousarray(ebT).astype(ml_dtypes.bfloat16),
            lawT=np.ascontiguousarray(lawT).astype(ml_dtypes.bfloat16),
            WT=WT.reshape(4, 128, HID).transpose(1, 0, 2).astype(
                ml_dtypes.bfloat16).copy(),
            ones128=ones128_np,
            E4=E4_np,
            ones128f=ones128f_np,
        ))
    return in_maps


def kernel(**inputs):
    in_maps = _prepare_in_maps(**inputs)
    nc = _get_program()
    res = run_bass_kernel_spmd(nc, in_maps, list(range(8)))

    out = np.empty((B, T, P, HID), dtype=np.float32)
    for c in range(8):
        b, th = c // 2, c % 2
        out[b, th * TQ:(th + 1) * TQ] = res.results[c]["out"].astype(np.float32)
    return out


# revision 21
# speedup vs baseline: 1.2067x; 1.0246x over previous
"""MemEffEquivariantAttention TRN2 Bass kernel (transposed-scores flow, v3).

Sharding: 8 cores = 4 batches x 2 query-token halves (fully data-parallel,
no collectives).

Scores are computed TRANSPOSED (wT[s, t] = kT_chunk.T @ qT) so the
attention probabilities already have s on partitions and feed the attn
matmul directly -- no transpose of probabilities is ever materialized.
Z (softmax denominator, per (head, t)) is recovered with ones-vector
matmuls over the s-partitions.

v4: Z-normalization is applied LATE: the main loop stashes UNNORMALIZED
attn rows into X (plain ACT copy), Z rows are collected per group of 4
heads into an SBUF tile, one DVE reciprocal per group computes rz, gpsimd
broadcasts it into an rzX tile, and 3 DVE multiplies per group normalize
X in place (sumsq follows on the normalized X).  This removes the
per-head reciprocal (1.7us each) / ACT ln-exp chain (which thrashed
activation tables at 1.3us per reload) from the critical path entirely.

v3 changes driven by the previous trace (137us: PE busy 86us with
439 small matmuls; DVE 81us of which 28us was `reciprocal` on
single-partition [1,256] tiles; 57us of HWDGE dma_start issue cost
spread over SP/ACT/Pool):
  - all heads use host-side exp(bias) (eb): the bias identity matmuls
    are gone (-32 matmuls); bias add is a bf16 DVE multiply.
  - 1/Z is computed as exp(-ln(Z)) on the ACT engine (ln and exp live in
    the same activation table set -> no table reloads), broadcast across
    partitions on the otherwise-idle gpsimd engine.  No DVE reciprocal.
  - q/k/bias loads are batched per group of 4 heads (3 dma_starts per
    group instead of 12), v/law/WT/out merged; X-stash DMAs split
    across the sync and gpsimd rings.
  - exp uses a constant -40 bias (softmax shift, folded out exactly by
    Z) to keep e/m0 in comfortable bf16/f32 range.
  - q/k fp16, output bf16 (upcast on host).
"""
import sys
sys.path.insert(0, "/opt/trn_rl_repo")

import numpy as np
import ml_dtypes

import concourse.bacc as bacc
import concourse.tile as tile
from concourse import mybir
from concourse.bass_utils import run_bass_kernel_spmd

F32 = mybir.dt.float32
F16 = mybir.dt.float16
BF16 = mybir.dt.bfloat16
AF = mybir.ActivationFunctionType

B, T, P, HID = 4, 512, 3, 512
HD, H = 32, 16
EXP, S = 512, 1024
TQ = 256            # query tokens per core
EPS = 1e-3
CUTOFF = 1e-5
NEG = -1e30
D = P * HD          # 96, per-head feature dim
SHIFT = -40.0       # constant softmax shift, cancels exactly via Z

_prog_cache = {}


def _build_program():
    nc = bacc.Bacc("TRN2", target_bir_lowering=False, debug=False)

    qT_d = nc.dram_tensor("qT", [H, D, TQ], F16, kind="ExternalInput").ap()
    kT_d = nc.dram_tensor("kT", [H, D, S], F16, kind="ExternalInput").ap()
    vS_d = nc.dram_tensor("vS", [128, 8, H, D], BF16, kind="ExternalInput").ap()
    # eb = exp(masked bias)^T, [H, s(part,chunk), t]
    eb_d = nc.dram_tensor("eb", [H, 128, 2, 4 * TQ], BF16, kind="ExternalInput").ap()
    lawT_d = nc.dram_tensor("lawT", [128, 2, 4 * TQ], BF16, kind="ExternalInput").ap()
    WT_d = nc.dram_tensor("WT", [128, 4, HID], BF16, kind="ExternalInput").ap()
    ones128_d = nc.dram_tensor("ones128", [128, 1], BF16, kind="ExternalInput").ap()
    E4_d = nc.dram_tensor("E4", [128, 4, 4], BF16, kind="ExternalInput").ap()
    ones128f_d = nc.dram_tensor("ones128f", [128, 1], F32, kind="ExternalInput").ap()
    out_d = nc.dram_tensor("out", [TQ, P, HID], BF16, kind="ExternalOutput").ap()

    with tile.TileContext(nc) as tc:
        with tc.tile_pool(name="const", bufs=1) as cp, \
             tc.tile_pool(name="kq", bufs=3) as kq, \
             tc.tile_pool(name="ebp", bufs=3) as ebp, \
             tc.tile_pool(name="eu", bufs=2) as eu, \
             tc.tile_pool(name="work", bufs=3) as wp, \
             tc.tile_pool(name="psw", bufs=2, space="PSUM") as psw, \
             tc.tile_pool(name="psa", bufs=2, space="PSUM") as psa, \
             tc.tile_pool(name="psz", bufs=2, space="PSUM") as psz:

            # ---- constants ----
            vS_t = cp.tile([128, 8, H, D], BF16, tag="vS")
            lawT_t = cp.tile([128, 2, 4 * TQ], BF16, tag="lawT")
            WT_t = cp.tile([128, 4, HID], BF16, tag="WT")
            ones128_t = cp.tile([128, 1], BF16, tag="o128")
            E4_t = cp.tile([128, 4, 4], BF16, tag="E4")
            ones128f_t = cp.tile([128, 1], F32, tag="o128f")
            eps_t = cp.tile([128, 1], F32, tag="eps")
            shift_t = cp.tile([128, 1], F32, tag="shift")
            X_t = cp.tile([128, P, 4, TQ], BF16, tag="X")
            rzX_t = cp.tile([128, 4, TQ], F32, tag="rzX")
            sqacc_t = cp.tile([128, TQ], F32, tag="sqacc")
            nc.vector.memset(eps_t[:], EPS)
            nc.vector.memset(shift_t[:], SHIFT)

            kT_tiles, qT_tiles, eb_tiles = {}, {}, {}
            m0_tiles, u_tiles, z4_tiles, at_tiles = {}, {}, {}, {}

            def emit_loads(g):
                """Load kT/qT/eb for the 4 heads of group g."""
                kT_t = kq.tile([D, 4, S], F16, tag="kT", name=f"kT_{g}")
                qT_t = kq.tile([D, 4, TQ], F16, tag="qT", name=f"qT_{g}")
                eb_t = ebp.tile([128, 4, 2, 4 * TQ], BF16, tag="eb",
                                name=f"eb_{g}")
                hs = slice(4 * g, 4 * g + 4)
                nc.sync.dma_start(out=kT_t[:],
                                  in_=kT_d[hs].rearrange("h d s -> d h s"))
                nc.sync.dma_start(out=qT_t[:],
                                  in_=qT_d[hs].rearrange("h d t -> d h t"))
                if g == 0:
                    # per-head chunks so head 0 can start ~2.5us earlier
                    for h4 in range(4):
                        nc.sync.dma_start(out=eb_t[:, h4, :, :],
                                          in_=eb_d[h4])
                else:
                    nc.sync.dma_start(out=eb_t[:],
                                      in_=eb_d[hs].rearrange("h p f x -> p h f x"))
                kT_tiles[g], qT_tiles[g], eb_tiles[g] = kT_t, qT_t, eb_t

            def emit_scores(h):
                g, h4 = h // 4, h % 4
                kT_t, qT_t, eb_t = kT_tiles[g], qT_tiles[g], eb_tiles[g]
                e_t = eu.tile([128, 2, 4 * TQ], BF16, tag="e", name=f"e_{h}")
                m0_t = eu.tile([128, 2, 4 * TQ], BF16, tag="m0",
                               name=f"m0_{h}")
                u_t = eu.tile([128, 2, 4 * TQ], BF16, tag="u", name=f"u_{h}")
                for hf in range(2):
                    w_ps = psw.tile([128, 4 * TQ], F32, tag="w",
                                    name=f"w_{h}_{hf}")
                    for sc4 in range(4):
                        sc = 4 * hf + sc4
                        nc.tensor.matmul(w_ps[:, sc4 * TQ:(sc4 + 1) * TQ],
                                         kT_t[:, h4, sc * 128:(sc + 1) * 128],
                                         qT_t[:, h4, :],
                                         start=True, stop=True,
                                         skip_group_check=True)
                    nc.scalar.activation(e_t[:, hf, :], w_ps[:], AF.Exp,
                                         bias=shift_t[:])
                    nc.vector.tensor_mul(m0_t[:, hf, :], e_t[:, hf, :],
                                         eb_t[:, h4, hf, :])
                    nc.vector.tensor_mul(u_t[:, hf, :], m0_t[:, hf, :],
                                         lawT_t[:, hf, :])
                m0_tiles[h], u_tiles[h] = m0_t, u_t

            def emit_post_pe(h):
                g, h4 = h // 4, h % 4
                m0_t, u_t = m0_tiles[h], u_tiles[h]
                if h4 == 0:
                    z4_tiles[g] = psz.tile([4, TQ], F32, tag="z",
                                           name=f"z4_{g}")
                z4_ps = z4_tiles[g]
                # one-hot lhsT: head h4 writes row h4, zeros elsewhere,
                # so the whole group accumulates into one [4, 256] tile
                for sc in range(8):
                    nc.tensor.matmul(z4_ps[:], E4_t[:, h4, :],
                                     m0_t[:, sc // 4,
                                          (sc % 4) * TQ:(sc % 4 + 1) * TQ],
                                     start=(h4 == 0 and sc == 0),
                                     stop=(h4 == 3 and sc == 7),
                                     skip_group_check=True)
                at_ps = psa.tile([D, TQ], F32, tag="at", name=f"at_{h}")
                for sc in range(8):
                    nc.tensor.matmul(at_ps[:], vS_t[:, sc, h, :],
                                     u_t[:, sc // 4,
                                         (sc % 4) * TQ:(sc % 4 + 1) * TQ],
                                     start=(sc == 0), stop=(sc == 7))
                at_tiles[h] = at_ps

            def emit_post_rest(h):
                del m0_tiles[h], u_tiles[h]
                at_ps = at_tiles.pop(h)
                at_sb = wp.tile([D, TQ], BF16, tag="atsb", name=f"atsb_{h}")
                nc.scalar.activation(at_sb[:], at_ps[:], AF.Copy)
                # stash into X[(h%4)*32+j, p, h//4, t] for out_proj lhsT
                for p in range(P):
                    nc.sync.dma_start(
                        out=X_t[(h % 4) * 32:(h % 4 + 1) * 32, p, h // 4, :],
                        in_=at_sb[p * 32:(p + 1) * 32, :])

            def emit_rz(g):
                # rz for the 4 heads of group g, in one reciprocal, then
                # replicate each rz4 row 32x across partitions via one
                # stride-0-source DMA: rzX[hm*32+j, g, :] = rz4[hm, :]
                rz4_t = wp.tile([4, TQ], F32, tag="rz4", name=f"rz4_{g}")
                nc.vector.reciprocal(rz4_t[:], z4_tiles.pop(g)[:])
                nc.scalar.dma_start(
                    out=rzX_t[:, g, :],
                    in_=rz4_t[:].unsqueeze(1).broadcast_to([4, 32, TQ]))

            def emit_xnorm_piece(g, p):
                # normalize one p-slice of X in place + its sumsq piece;
                # spread one piece per head so the in-order DVE queue is
                # never jammed by a group-boundary burst
                nc.vector.tensor_mul(X_t[:, p, g, :], X_t[:, p, g, :],
                                     rzX_t[:, g, :])
                if g == 0 and p == 0:
                    nc.vector.tensor_mul(sqacc_t[:], X_t[:, p, g, :],
                                         X_t[:, p, g, :])
                else:
                    sq_t = wp.tile([128, TQ], BF16, tag="sq")
                    nc.vector.tensor_mul(sq_t[:], X_t[:, p, g, :],
                                         X_t[:, p, g, :])
                    nc.vector.tensor_add(sqacc_t[:], sqacc_t[:], sq_t[:])

            # ---- main loop, software-pipelined by one head ----
            emit_loads(0)
            nc.scalar.dma_start(out=lawT_t[:], in_=lawT_d)
            nc.scalar.dma_start(out=ones128_t[:], in_=ones128_d)
            nc.scalar.dma_start(out=E4_t[:], in_=E4_d)
            nc.scalar.dma_start(out=ones128f_t[:], in_=ones128f_d)
            emit_loads(1)
            for half in range(2):
                nc.scalar.dma_start(out=vS_t[:, 4 * half:4 * half + 4, :, :],
                                    in_=vS_d[:, 4 * half:4 * half + 4, :, :])
            nc.scalar.dma_start(out=WT_t[:], in_=WT_d)

            pieces = []
            for h in range(H):
                if h % 4 == 0 and h // 4 + 2 < 4:
                    emit_loads(h // 4 + 2)
                emit_scores(h)
                if h >= 1:
                    emit_post_pe(h - 1)
                    emit_post_rest(h - 1)
                if h % 4 == 1 and h >= 5:
                    emit_rz(h // 4 - 1)
                    pieces += [(h // 4 - 1, p) for p in range(P)]
                elif pieces:
                    emit_xnorm_piece(*pieces.pop(0))
            emit_post_pe(H - 1)
            emit_post_rest(H - 1)
            emit_rz(3)
            for gp in pieces + [(3, p) for p in range(P)]:
                emit_xnorm_piece(*gp)

            # ---- inv = 1/sqrt(mean+eps), out_proj, scale, store ----
            ss_ps = [psz.tile([128, 1], F32, tag="z", name=f"ss{tb}")
                     for tb in range(2)]
            for tb in range(2):
                nc.tensor.matmul(ss_ps[tb][:],
                                 sqacc_t[:, tb * 128:(tb + 1) * 128],
                                 ones128f_t[:], start=True, stop=True)
            inv_t = []
            for tb in range(2):
                tmp_t = wp.tile([128, 1], F32, tag=f"tmp{tb}")
                nc.scalar.activation(tmp_t[:], ss_ps[tb][:], AF.Sqrt,
                                     scale=1.0 / HID, bias=eps_t[:])
                iv = wp.tile([128, 1], F32, tag=f"inv{tb}")
                nc.vector.reciprocal(iv[:], tmp_t[:])
                inv_t.append(iv)

            for tb in range(2):
                o_sb = wp.tile([128, P, HID], BF16, tag="osb",
                               name=f"osb_{tb}")
                for p in range(P):
                    o_ps = psw.tile([128, HID], F32, tag="w",
                                    name=f"o_{p}_{tb}")
                    for ci in range(4):
                        nc.tensor.matmul(o_ps[:],
                                         X_t[:, p, ci, tb * 128:(tb + 1) * 128],
                                         WT_t[:, ci, :],
                                         start=(ci == 0), stop=(ci == 3))
                    nc.vector.tensor_scalar_mul(o_sb[:, p, :], o_ps[:],
                                                inv_t[tb][:])
                nc.sync.dma_start(
                    out=out_d[tb * 128:(tb + 1) * 128, :, :], in_=o_sb[:])

    nc.compile()
    return nc


def _get_program():
    if "nc" not in _prog_cache:
        _prog_cache["nc"] = _build_program()
    return _prog_cache["nc"]


def _prepare_in_maps(q, k, v, attn_bias, key_padding_mask, outcell_index,
                     local_attention_weight, expand_mask, out_proj_weight,
                     attn_ln_weight):
    q = np.asarray(q, dtype=np.float32)
    k = np.asarray(k, dtype=np.float32)
    v = np.asarray(v, dtype=np.float32)
    attn_bias = np.asarray(attn_bias, dtype=np.float32)
    kpm = np.asarray(key_padding_mask)
    idx = np.asarray(outcell_index).astype(np.int64)
    law = np.asarray(local_attention_weight, dtype=np.float32)
    emask = np.asarray(expand_mask)
    W = np.asarray(out_proj_weight, dtype=np.float32)
    lnw = np.asarray(attn_ln_weight, dtype=np.float32)

    WT = np.ascontiguousarray((W * lnw[None, :]).T)  # [hid, o], ln folded
    ones128_np = np.ones((128, 1), dtype=ml_dtypes.bfloat16)
    E4_np = np.zeros((128, 4, 4), dtype=ml_dtypes.bfloat16)
    for i in range(4):
        E4_np[:, i, i] = 1
    ones128f_np = np.ones((128, 1), dtype=np.float32)

    in_maps = []
    for c in range(8):
        b, th = c // 2, c % 2
        tsl = slice(th * TQ, (th + 1) * TQ)

        # kT [H, 96, S]: kf[s, p, h*32+hd] with s-expansion host-gathered
        kf = np.concatenate([k[b], k[b][idx[b]]], axis=0)  # [S, P, HID]
        kT = kf.reshape(S, P, H, HD).transpose(2, 1, 3, 0).reshape(H, D, S)
        qT = q[b, tsl].reshape(TQ, P, H, HD).transpose(2, 1, 3, 0) \
            .reshape(H, D, TQ)

        # vS [128, 8, H, 96]: vS[part, sc, h, (p,hd)] = vf[sc*128+part, ...]
        vf = np.concatenate([v[b], v[b][idx[b]]], axis=0)  # [S, P, HID]
        vS = vf.reshape(8, 128, P, H, HD).transpose(1, 0, 3, 2, 4) \
            .reshape(128, 8, H, D)

        # masked bias [H, 256, S]
        bias_c = np.ascontiguousarray(attn_bias[b, :, tsl, :])
        kpmS = np.concatenate([kpm[b], emask[b]])           # [S]
        if kpmS.any():
            bias_c[:, :, kpmS] = NEG
        cut = law[b, tsl] <= CUTOFF                         # [256, S]
        if cut.any():
            bias_c[:, cut] = NEG
        # exp, transpose to [H, S, 256] -> [H, 128, 2, 1024]
        ebT = np.exp(bias_c.transpose(0, 2, 1)).reshape(H, 8, 128, TQ) \
            .transpose(0, 2, 1, 3).reshape(H, 128, 2, 4 * TQ)

        lawT = law[b, tsl].T.reshape(8, 128, TQ).transpose(1, 0, 2) \
            .reshape(128, 2, 4 * TQ)

        in_maps.append(dict(
            qT=qT.astype(np.float16),
            kT=np.ascontiguousarray(kT).astype(np.float16),
            vS=np.ascontiguousarray(vS).astype(ml_dtypes.bfloat16),
            eb=np.ascontiguousarray(ebT).astype(ml_dtypes.bfloat16),
            lawT=np.ascontiguousarray(lawT).astype(ml_dtypes.bfloat16),
            WT=WT.reshape(4, 128, HID).transpose(1, 0, 2).astype(
                ml_dtypes.bfloat16).copy(),
            ones128=ones128_np,
            E4=E4_np,
            ones128f=ones128f_np,
        ))
    return in_maps


def kernel(**inputs):
    in_maps = _prepare_in_maps(**inputs)
    nc = _get_program()
    res = run_bass_kernel_spmd(nc, in_maps, list(range(8)))

    out = np.empty((B, T, P, HID), dtype=np.float32)
    for c in range(8):
        b, th = c // 2, c % 2
        out[b, th * TQ:(th + 1) * TQ] = res.results[c]["out"].astype(np.float32)
    return out
